# revision 7
# baseline (speedup 1.0000x reference)
"""S4ND Darcy-flow Bass kernel v3: builder + host-side preparation.

Design (per core = one batch element, batch-parallel over 4 cores, cores
4..7 duplicate work and are ignored at gather time):

  state h_sb: SBUF bf16 [128p=h, (w, d)], d innermost.
  Per layer, per half hf (128 channels each):
    stage A (conv), per 2-pair group (4 channels), one 8-bank psum pool:
      slot1 <- MM1 x4: A^T[w, h'] = U_d^T @ ThT_d
      copy1 (ACT, FD=512): slot1 -> At4 bf16
      slot2 <- MM2 x4 + MMd x4 (accumulate):
               y[h', w'] = At_d @ TwT_d + (D_d I) @ U_d   (D-skip on PE,
               dI tiles shipped from host)
      Ycopy (DVE, FD=512): slot2 -> Ysb[:, w, 4ch] (w-major [128, W, DH])
    stage B, per 4-w0 quad:
      DMA transpose x4: Ysb[:, w0+i, :] -> stg [dm, 4, h]  (DMA xbar engine)
      gelu (ACT, FD=512): stg -> Xt[hf]
  stage C, per w0 (8-deep psum pipeline):
    W_out GEMM x2 (k chunks) into psW [128, 512]
    tanh (ACT): t1 = tanh(0.5*g)         [sigmoid via tanh: same act table]
    glu (DVE stt + accum): glu = (t1+1)*a', accum ssum  (for layers >= 1,
        sum_d h == 0 exactly since h is LayerNorm output, so sum_d p =
        sum_d glu; layer 0 accumulates on the residual op instead)
    sumsq (DVE/ACT alternating, + accum sqs)
  residual (DVE tt, batched FD=2048 per 8 w0): h_sb += glu_big
  stats (batched, per layer): mu/var (DVE), std=sqrt(var+eps) (ACT, one
    table switch), rr=1/std (DVE recip), nmr=-mu*rr (DVE)
  normalize, per w0 (DVE/ACT alternating, per-partition AP scalars):
    h_sb = p*rr + nmr
  Decoder: DVE stt dot-products per w slice -> out (h, w) f32.

Host precomputes (numpy, float64): S4D kernels kh/kw, transposed Toeplitz
matrices ThT/TwT, D*I diagonal tiles, replicated small tensors, xg packing.
W_out a-half is pre-scaled by 0.5 for the tanh-based GLU.
"""

import numpy as np
import ml_dtypes

import concourse.bacc as bacc
import concourse.mybir as mybir
import concourse.tile as tile

bf16 = ml_dtypes.bfloat16
AF = mybir.ActivationFunctionType
OP = mybir.AluOpType
F32 = mybir.dt.float32
BF = mybir.dt.bfloat16

H = 128
W = 128


def host_prep(inputs, n_layers=None, d_model=None):
    """Compute device-side constant tensors from the full model inputs."""
    log_dt = np.asarray(inputs["log_dt"], np.float64)     # (L,2,d)
    logA_re = np.asarray(inputs["logA_re"], np.float64)   # (L,2,d,N)
    A_im = np.asarray(inputs["A_im"], np.float64)
    C_re = np.asarray(inputs["C_re"], np.float64)
    C_im = np.asarray(inputs["C_im"], np.float64)
    Dskip = np.asarray(inputs["Dskip"], np.float64)       # (L,d)
    W_out = np.asarray(inputs["W_out"], np.float64)       # (L,d,2d)
    b_out = np.asarray(inputs["b_out"], np.float64)       # (L,2d)
    ln_w = np.asarray(inputs["ln_w"], np.float64)         # (L,d)
    ln_b = np.asarray(inputs["ln_b"], np.float64)
    W_enc = np.asarray(inputs["W_enc"], np.float64)       # (2,d)
    b_enc = np.asarray(inputs["b_enc"], np.float64)       # (d,)
    W_dec = np.asarray(inputs["W_dec"], np.float64)       # (d,1)
    b_dec = np.asarray(inputs["b_dec"], np.float64)       # (1,)
    x = np.asarray(inputs["x"], np.float32)               # (B,H,W,1)
    grid = np.asarray(inputs["grid"], np.float32)

    L = log_dt.shape[0] if n_layers is None else n_layers
    D = log_dt.shape[2] if d_model is None else d_model
    log_dt = log_dt[:L, :, :D]
    logA_re = logA_re[:L, :, :D]
    A_im = A_im[:L, :, :D]
    C_re = C_re[:L, :, :D]
    C_im = C_im[:L, :, :D]
    Dskip = Dskip[:L, :D]
    d_full = W_out.shape[1]
    Wa = W_out[:L, :D, :D] * 0.5          # pre-scale a-half for tanh GLU
    Wg = W_out[:L, :D, d_full:d_full + D]
    W_out2 = np.concatenate([Wa, Wg], axis=2)             # (L, D, 2D)
    b_out2 = np.concatenate([b_out[:L, :D] * 0.5,
                             b_out[:L, d_full:d_full + D]], axis=1)
    ln_w = ln_w[:L, :D]
    ln_b = ln_b[:L, :D]
    W_enc = W_enc[:, :D]
    b_enc = b_enc[:D]
    W_dec = W_dec[:D]

    # ---- S4D kernels ----
    dt = np.exp(log_dt)[..., None]                        # (L,2,D,1)
    A = -np.exp(logA_re) + 1j * A_im                      # (L,2,D,N)
    C = C_re + 1j * C_im
    dtA = dt * A
    CB = C * (np.exp(dtA) - 1.0) / A
    t = np.arange(H, dtype=np.float64)
    pows = np.exp(dtA[..., None] * t)                     # (L,2,D,N,H)
    K = 2.0 * np.real(np.einsum("lxdn,lxdnt->lxdt", CB, pows))  # (L,2,D,H)
    kh = K[:, 0]                                          # (L,D,H)
    kw = K[:, 1]                                          # (L,D,W)

    # transposed lower-triangular Toeplitz: ThT[l,d,i,p] = kh[l,d,p-i], p>=i
    idx = np.arange(H)[None, :] - np.arange(H)[:, None]   # (i,p) = p-i
    mask = idx >= 0
    idxc = np.clip(idx, 0, H - 1)
    ThT = np.where(mask, kh[:, :, idxc], 0.0)             # (L,D,128,128)
    TwT = np.where(mask, kw[:, :, idxc], 0.0)

    # D-skip diagonal tiles: dii[l,d] = D[l,d] * I  (bf16, for PE MMd)
    eye = np.eye(128, dtype=np.float64)
    dii = (Dskip[:, :, None, None] * eye[None, None]).astype(np.float32)

    flags = dict(
        use_ln_affine=not (np.all(ln_w == 1.0) and np.all(ln_b == 0.0)),
        use_b_out=not np.all(b_out2 == 0.0),
        n_layers=L,
        d_model=D,
        b_dec=float(b_dec[0]),
    )

    common = dict(
        wenc=W_enc.astype(np.float32).astype(bf16),                       # (2,D)
        benc_rep=np.tile(b_enc.astype(np.float32)[None, :], (128, 1)),    # (128,D) f32
        tht=ThT.astype(np.float32).astype(bf16),                          # (L,D,128,128)
        twt=TwT.astype(np.float32).astype(bf16),
        dii=dii.astype(bf16),                                             # (L,D,128,128)
        wdec_rep=np.tile(W_dec.astype(np.float32).reshape(1, D), (128, 1)).astype(bf16),
        ident=np.eye(128, dtype=np.float32).astype(bf16),
    )
    nk = max(1, D // 128)
    common["wout"] = np.ascontiguousarray(
        W_out2.reshape(L, nk, min(D, 128), 2 * D).astype(np.float32).astype(bf16)
    )
    if flags["use_ln_affine"]:
        common["lnw_rep"] = np.tile(ln_w.astype(np.float32)[:, None, :], (1, 128, 1)).astype(bf16)
        common["lnb_rep"] = np.tile(ln_b.astype(np.float32)[:, None, :], (1, 128, 1)).astype(bf16)
    if flags["use_b_out"]:
        common["bout_rep"] = np.tile(b_out2.astype(np.float32)[:, None, :], (1, 128, 1))

    per_batch = []
    for b in range(x.shape[0]):
        # xg[0, w*128+h] = x[b,h,w];  xg[1,...] = grid
        xb = x[b, :, :, 0].T.reshape(-1)     # (w,h) order
        gb = grid[b, :, :, 0].T.reshape(-1)
        xg = np.stack([xb, gb], axis=0).astype(np.float32).astype(bf16)
        per_batch.append(dict(xg=xg))
    return common, per_batch, flags


def build_program(flags, num_devices=8, gelu_fn=None):
    """Emit the bass program. Returns the compiled Bacc."""
    L = flags["n_layers"]
    D = flags["d_model"]
    DH = D // 2            # channels per half
    NK = max(1, D // 128)  # K tiles in W_out GEMM
    assert D % 2 == 0

    if gelu_fn is None:
        gelu_fn = AF.Gelu_apprx_tanh
    nc = bacc.Bacc("TRN2", target_bir_lowering=False, debug=False,
                   num_devices=num_devices)

    def din(name, shape, dt):
        return nc.dram_tensor(name, shape, dt, kind="ExternalInput").ap()

    xg = din("xg", [2, H * W], BF)
    wenc = din("wenc", [2, D], BF)
    benc_rep = din("benc_rep", [128, D], F32)
    tht = din("tht", [L, D, 128, 128], BF)
    twt = din("twt", [L, D, 128, 128], BF)
    dii = din("dii", [L, D, 128, 128], BF)
    wout = din("wout", [L, NK, min(D, 128), 2 * D], BF)
    wdec_rep = din("wdec_rep", [128, D], BF)
    ident = din("ident", [128, 128], BF)
    if flags["use_ln_affine"]:
        lnw_rep = din("lnw_rep", [L, 128, D], BF)
        lnb_rep = din("lnb_rep", [L, 128, D], BF)
    if flags["use_b_out"]:
        bout_rep = din("bout_rep", [L, 128, 2 * D], F32)
    out = nc.dram_tensor("out", [H, W], F32, kind="ExternalOutput").ap()

    from contextlib import ExitStack
    with tile.TileContext(nc) as tc, ExitStack() as ctx:
        state = ctx.enter_context(tc.tile_pool(name="state", bufs=1))
        consts = ctx.enter_context(tc.tile_pool(name="consts", bufs=1))
        wring = ctx.enter_context(tc.tile_pool(name="wring", bufs=12))
        lring = ctx.enter_context(tc.tile_pool(name="lring", bufs=2))
        atring = ctx.enter_context(tc.tile_pool(name="atring", bufs=3))
        cring = ctx.enter_context(tc.tile_pool(name="cring", bufs=3))
        gring = ctx.enter_context(tc.tile_pool(name="gring", bufs=2))
        sring = ctx.enter_context(tc.tile_pool(name="sring", bufs=3))
        stats = ctx.enter_context(tc.tile_pool(name="stats", bufs=1))
        ps = ctx.enter_context(tc.tile_pool(name="ps", bufs=8, space="PSUM"))

        h_sb = state.tile([128, W, D], BF, tag="h")
        Ysb = state.tile([128, W, DH], BF, tag="y")
        Xts = [state.tile([128, H * W], BF, tag=f"xt{k}", name=f"xt{k}")
               for k in range(NK)]

        wenc_t = consts.tile([2, D], BF)
        nc.sync.dma_start(wenc_t[:], wenc[:])
        benc_t = consts.tile([128, D], F32)
        nc.sync.dma_start(benc_t[:], benc_rep[:])
        ident_t = consts.tile([128, 128], BF)
        nc.sync.dma_start(ident_t[:], ident[:])
        wdec_t = consts.tile([128, D], BF)
        nc.sync.dma_start(wdec_t[:], wdec_rep[:])
        eps_t = consts.tile([128, 1], F32)
        nc.vector.memset(eps_t[:], 1e-5)

        # ---------------- encoder ----------------
        for w0 in range(W):
            xg_t = wring.tile([2, 128], BF, tag="xg")
            nc.sync.dma_start(xg_t[:], xg[:, w0 * 128:(w0 + 1) * 128])
            psE = ps.tile([128, 2 * D], F32, tag="ps")
            nc.tensor.matmul(psE[:, 0:D], xg_t[:], wenc_t[:],
                             start=True, stop=True)
            nc.vector.scalar_tensor_tensor(
                out=h_sb[:, w0, :], in0=psE[:, 0:D], scalar=1.0,
                in1=benc_t[:], op0=OP.mult, op1=OP.add)

        # ---------------- layers ----------------
        for l in range(L):
            wout_ts = []
            for k in range(NK):
                wt = lring.tile([min(D, 128), 2 * D], BF, tag="woutw")
                nc.sync.dma_start(wt[:], wout[l, k])
                wout_ts.append(wt)
            if flags["use_ln_affine"]:
                lnw_t = lring.tile([128, D], BF, tag="lnw")
                nc.sync.dma_start(lnw_t[:], lnw_rep[l])
                lnb_t = lring.tile([128, D], BF, tag="lnb")
                nc.sync.dma_start(lnb_t[:], lnb_rep[l])
            if flags["use_b_out"]:
                bout_t = lring.tile([128, 2 * D], F32, tag="bout")
                nc.sync.dma_start(bout_t[:], bout_rep[l])

            for hf in range(2):
                # ---- stage A: convolutions, 2 pairs (4 channels) at a time
                for dm in range(0, DH, 4):
                    d = hf * DH + dm
                    thts, twts, diis = [], [], []
                    for j in range(4):
                        tt_ = wring.tile([128, 128], BF, tag="tht")
                        nc.sync.dma_start(tt_[:], tht[l, d + j])
                        thts.append(tt_)
                        tw_ = wring.tile([128, 128], BF, tag="twt")
                        nc.sync.dma_start(tw_[:], twt[l, d + j])
                        twts.append(tw_)
                        di_ = wring.tile([128, 128], BF, tag="dii")
                        nc.sync.dma_start(di_[:], dii[l, d + j])
                        diis.append(di_)

                    slot1 = ps.tile([128, 4, 128], F32, tag="ps")
                    for j in range(4):
                        nc.tensor.matmul(slot1[:, j, :], h_sb[:, :, d + j],
                                         thts[j][:], start=True, stop=True)
                    At4 = atring.tile([128, 4, 128], BF, tag="at")
                    nc.scalar.copy(At4[:], slot1[:])

                    slot2 = ps.tile([128, 4, 128], F32, tag="ps")
                    for j in range(4):
                        nc.tensor.matmul(slot2[:, j, :], At4[:, j, :],
                                         twts[j][:], start=True, stop=False)
                        nc.tensor.matmul(slot2[:, j, :], diis[j][:],
                                         h_sb[:, :, d + j],
                                         start=False, stop=True)
                    # copy-out to Ysb (w-major): dst AP [p, c, w]
                    nc.vector.tensor_copy(
                        Ysb[:, :, dm:dm + 4].rearrange("p w c -> p c w"),
                        slot2[:])

                # ---- stage B: DMA transpose + gelu into Xt ----
                for w0 in range(0, W, 4):
                    stg = sring.tile([128, 4, 128], BF, tag="stg")
                    for i in range(4):
                        nc.sync.dma_start_transpose(stg[:, i, :],
                                                    Ysb[:, w0 + i, :])
                    nc.scalar.activation(
                        Xts[hf][:, w0 * 128:(w0 + 4) * 128],
                        stg[:].rearrange("p a b -> p (a b)"), gelu_fn)

            # ---- stage C: W_out GEMM + GLU + residual + stats ----
            ssum = stats.tile([128, W], F32, tag="ssum")
            sqs = stats.tile([128, W], F32, tag="sqs")
            mu = stats.tile([128, W], F32, tag="mu")
            var = stats.tile([128, W], F32, tag="var")
            std = stats.tile([128, W], F32, tag="std")
            rr = stats.tile([128, W], F32, tag="rr")
            nmr = stats.tile([128, W], F32, tag="nmr")
            RB = 8  # residual batch (w0 per batched residual add)
            glu_big = None
            for wv in range(W):
                if wv % RB == 0:
                    glu_big = gring.tile([128, RB, D], BF, tag="glu")
                psW = ps.tile([128, 2 * D], F32, tag="ps")
                for k in range(NK):
                    nc.tensor.matmul(
                        psW[:],
                        Xts[k][:, wv * 128:(wv + 1) * 128],
                        wout_ts[k][:], start=(k == 0),
                        stop=(k == NK - 1))
                if flags["use_b_out"]:
                    nc.vector.tensor_tensor(psW[:], psW[:], bout_t[:],
                                            op=OP.add)
                # t1 = tanh(g/2); glu = (t1+1)*a'  (a' = 0.5*a baked in W_out)
                t1 = cring.tile([128, D], BF, tag="t1")
                nc.scalar.activation(t1[:], psW[:, D:2 * D], AF.Tanh,
                                     scale=0.5)
                g_sl = glu_big[:, wv % RB, :]
                if l > 0:
                    # sum_d h == 0 (LN output) => ssum accumulates on glu
                    nc.vector.scalar_tensor_tensor(
                        out=g_sl, in0=t1[:], scalar=1.0, in1=psW[:, 0:D],
                        op0=OP.add, op1=OP.mult,
                        accum_out=ssum[:, wv:wv + 1])
                else:
                    nc.vector.scalar_tensor_tensor(
                        out=g_sl, in0=t1[:], scalar=1.0, in1=psW[:, 0:D],
                        op0=OP.add, op1=OP.mult)
                if wv % RB == RB - 1:
                    b0 = wv - RB + 1
                    if l > 0:
                        # batched residual: h += glu
                        nc.vector.tensor_tensor(
                            h_sb[:, b0:wv + 1, :].rearrange("p a b -> p (a b)"),
                            glu_big[:].rearrange("p a b -> p (a b)"),
                            h_sb[:, b0:wv + 1, :].rearrange("p a b -> p (a b)"),
                            op=OP.add)
                    else:
                        for wx in range(b0, wv + 1):
                            nc.vector.scalar_tensor_tensor(
                                out=h_sb[:, wx, :], in0=glu_big[:, wx % RB, :],
                                scalar=1.0, in1=h_sb[:, wx, :],
                                op0=OP.mult, op1=OP.add,
                                accum_out=ssum[:, wx:wx + 1])
                    # sumsq per w0 (alternate DVE/ACT)
                    for wx in range(b0, wv + 1):
                        scr = cring.tile([128, D], BF, tag="scr")
                        if wx % 2 == 0:
                            nc.vector.scalar_tensor_tensor(
                                out=scr[:], in0=h_sb[:, wx, :], scalar=1.0,
                                in1=h_sb[:, wx, :], op0=OP.mult, op1=OP.mult,
                                accum_out=sqs[:, wx:wx + 1])
                        else:
                            nc.scalar.activation(
                                scr[:], h_sb[:, wx, :], AF.Square,
                                accum_out=sqs[:, wx:wx + 1])

            # ---- batched stats ----
            nc.vector.tensor_scalar(out=mu[:], in0=ssum[:],
                                    scalar1=1.0 / D, scalar2=None,
                                    op0=OP.mult)
            nc.vector.tensor_tensor(var[:], mu[:], mu[:], op=OP.mult)
            nc.vector.scalar_tensor_tensor(
                out=var[:], in0=sqs[:], scalar=1.0 / D,
                in1=var[:], op0=OP.mult, op1=OP.subtract)
            nc.scalar.activation(std[:], var[:], AF.Sqrt, bias=eps_t[:, 0:1])
            nc.vector.reciprocal(rr[:], std[:])
            nc.vector.scalar_tensor_tensor(
                out=nmr[:], in0=mu[:], scalar=-1.0,
                in1=rr[:], op0=OP.mult, op1=OP.mult)

            # ---- normalize pass (alternate DVE/ACT) ----
            for w0 in range(W):
                if w0 % 2 == 0:
                    nc.vector.tensor_scalar(
                        out=h_sb[:, w0, :], in0=h_sb[:, w0, :],
                        scalar1=rr[:, w0:w0 + 1], scalar2=nmr[:, w0:w0 + 1],
                        op0=OP.mult, op1=OP.add)
                else:
                    nc.scalar.activation(
                        h_sb[:, w0, :], h_sb[:, w0, :], AF.Identity,
                        bias=nmr[:, w0:w0 + 1], scale=rr[:, w0:w0 + 1])
                if flags["use_ln_affine"]:
                    nc.vector.tensor_tensor(
                        h_sb[:, w0, :], h_sb[:, w0, :], lnw_t[:],
                        op=OP.mult)
                    nc.vector.tensor_tensor(
                        h_sb[:, w0, :], h_sb[:, w0, :], lnb_t[:],
                        op=OP.add)

        # ---------------- decoder ----------------
        dec_sb = consts.tile([128, W], F32)
        for w0 in range(W):
            scr = cring.tile([128, D], BF, tag="scr")
            nc.vector.scalar_tensor_tensor(
                out=scr[:], in0=h_sb[:, w0, :], scalar=1.0, in1=wdec_t[:],
                op0=OP.mult, op1=OP.mult, accum_out=dec_sb[:, w0:w0 + 1])
        if flags["b_dec"] != 0.0:
            nc.vector.tensor_scalar(out=dec_sb[:], in0=dec_sb[:],
                                    scalar1=float(flags["b_dec"]), scalar2=None,
                                    op0=OP.add)
        nc.sync.dma_start(out[:], dec_sb[:])

    nc.compile()
    return nc


# ---------------------------------------------------------------------------
# Self-contained entry point: full inputs in, full output out.
# Shards batch-parallel across 8 NeuronCores (cores 4..7 duplicate work).
# ---------------------------------------------------------------------------

_PROGRAM_CACHE = {}


def _get_program(flags):
    key = (flags["n_layers"], flags["d_model"], flags["use_ln_affine"],
           flags["use_b_out"], flags["b_dec"])
    if key not in _PROGRAM_CACHE:
        _PROGRAM_CACHE[key] = build_program(flags, num_devices=8)
    return _PROGRAM_CACHE[key]


def kernel(**inputs):
    import os
    from concourse.bass_utils import run_bass_kernel_spmd

    common, per_batch, flags = host_prep(inputs)
    nc = _get_program(flags)

    B = len(per_batch)
    in_maps = []
    for c in range(8):
        m = dict(common)
        m.update(per_batch[c % B])
        in_maps.append(m)

    trace = bool(os.environ.get("S4ND_TRACE"))
    res = run_bass_kernel_spmd(nc, in_maps, core_ids=list(range(8)), trace=trace)
    if trace and res.exec_time_ns is not None:
        print(f"HW exec time: {res.exec_time_ns} ns")
        kernel.last_exec_time_ns = res.exec_time_ns
        kernel.last_results = res

    out = np.stack([res.results[b]["out"] for b in range(B)], axis=0)[..., None]
    return out.astype(np.float32)


# revision 9
# speedup vs baseline: 1.5385x; 1.5385x over previous
"""S4ND Darcy-flow Bass kernel v3: builder + host-side preparation.

Design (per core = one batch element, batch-parallel over 4 cores, cores
4..7 duplicate work and are ignored at gather time):

  state h_sb: SBUF bf16 [128p=h, (w, d)], d innermost.
  Per layer, per half hf (128 channels each):
    stage A (conv), per 2-pair group (4 channels), one 8-bank psum pool:
      slot1 <- MM1 x4: A^T[w, h'] = U_d^T @ ThT_d
      copy1 (ACT, FD=512): slot1 -> At4 bf16
      slot2 <- MM2 x4 + MMd x4 (accumulate):
               y[h', w'] = At_d @ TwT_d + (D_d I) @ U_d   (D-skip on PE,
               dI tiles shipped from host)
      Ycopy (DVE, FD=512): slot2 -> Ysb[:, w, 4ch] (w-major [128, W, DH])
    stage B, per 4-w0 quad:
      DMA transpose x4: Ysb[:, w0+i, :] -> stg [dm, 4, h]  (DMA xbar engine)
      gelu (ACT, FD=512): stg -> Xt[hf]
  stage C, per w0 (8-deep psum pipeline):
    W_out GEMM x2 (k chunks) into psW [128, 512]
    tanh (ACT): t1 = tanh(0.5*g)         [sigmoid via tanh: same act table]
    glu (DVE stt + accum): glu = (t1+1)*a', accum ssum  (for layers >= 1,
        sum_d h == 0 exactly since h is LayerNorm output, so sum_d p =
        sum_d glu; layer 0 accumulates on the residual op instead)
    sumsq (DVE/ACT alternating, + accum sqs)
  residual (DVE tt, batched FD=2048 per 8 w0): h_sb += glu_big
  stats (batched, per layer): mu/var (DVE), std=sqrt(var+eps) (ACT, one
    table switch), rr=1/std (DVE recip), nmr=-mu*rr (DVE)
  normalize, per w0 (DVE/ACT alternating, per-partition AP scalars):
    h_sb = p*rr + nmr
  Decoder: DVE stt dot-products per w slice -> out (h, w) f32.

Host precomputes (numpy, float64): S4D kernels kh/kw, transposed Toeplitz
matrices ThT/TwT, D*I diagonal tiles, replicated small tensors, xg packing.
W_out a-half is pre-scaled by 0.5 for the tanh-based GLU.
"""

import numpy as np
import ml_dtypes

import concourse.bacc as bacc
import concourse.mybir as mybir
import concourse.tile as tile

bf16 = ml_dtypes.bfloat16
AF = mybir.ActivationFunctionType
OP = mybir.AluOpType
F32 = mybir.dt.float32
BF = mybir.dt.bfloat16

H = 128
W = 128


def host_prep(inputs, n_layers=None, d_model=None):
    """Compute device-side constant tensors from the full model inputs."""
    log_dt = np.asarray(inputs["log_dt"], np.float64)     # (L,2,d)
    logA_re = np.asarray(inputs["logA_re"], np.float64)   # (L,2,d,N)
    A_im = np.asarray(inputs["A_im"], np.float64)
    C_re = np.asarray(inputs["C_re"], np.float64)
    C_im = np.asarray(inputs["C_im"], np.float64)
    Dskip = np.asarray(inputs["Dskip"], np.float64)       # (L,d)
    W_out = np.asarray(inputs["W_out"], np.float64)       # (L,d,2d)
    b_out = np.asarray(inputs["b_out"], np.float64)       # (L,2d)
    ln_w = np.asarray(inputs["ln_w"], np.float64)         # (L,d)
    ln_b = np.asarray(inputs["ln_b"], np.float64)
    W_enc = np.asarray(inputs["W_enc"], np.float64)       # (2,d)
    b_enc = np.asarray(inputs["b_enc"], np.float64)       # (d,)
    W_dec = np.asarray(inputs["W_dec"], np.float64)       # (d,1)
    b_dec = np.asarray(inputs["b_dec"], np.float64)       # (1,)
    x = np.asarray(inputs["x"], np.float32)               # (B,H,W,1)
    grid = np.asarray(inputs["grid"], np.float32)

    L = log_dt.shape[0] if n_layers is None else n_layers
    D = log_dt.shape[2] if d_model is None else d_model
    log_dt = log_dt[:L, :, :D]
    logA_re = logA_re[:L, :, :D]
    A_im = A_im[:L, :, :D]
    C_re = C_re[:L, :, :D]
    C_im = C_im[:L, :, :D]
    Dskip = Dskip[:L, :D]
    d_full = W_out.shape[1]
    Wa = W_out[:L, :D, :D] * 0.5          # pre-scale a-half for tanh GLU
    Wg = W_out[:L, :D, d_full:d_full + D]
    W_out2 = np.concatenate([Wa, Wg], axis=2)             # (L, D, 2D)
    b_out2 = np.concatenate([b_out[:L, :D] * 0.5,
                             b_out[:L, d_full:d_full + D]], axis=1)
    ln_w = ln_w[:L, :D]
    ln_b = ln_b[:L, :D]
    W_enc = W_enc[:, :D]
    b_enc = b_enc[:D]
    W_dec = W_dec[:D]

    # ---- S4D kernels ----
    dt = np.exp(log_dt)[..., None]                        # (L,2,D,1)
    A = -np.exp(logA_re) + 1j * A_im                      # (L,2,D,N)
    C = C_re + 1j * C_im
    dtA = dt * A
    CB = C * (np.exp(dtA) - 1.0) / A
    t = np.arange(H, dtype=np.float64)
    pows = np.exp(dtA[..., None] * t)                     # (L,2,D,N,H)
    K = 2.0 * np.real(np.einsum("lxdn,lxdnt->lxdt", CB, pows))  # (L,2,D,H)
    kh = K[:, 0]                                          # (L,D,H)
    kw = K[:, 1]                                          # (L,D,W)

    # transposed lower-triangular Toeplitz: ThT[l,d,i,p] = kh[l,d,p-i], p>=i
    idx = np.arange(H)[None, :] - np.arange(H)[:, None]   # (i,p) = p-i
    mask = idx >= 0
    idxc = np.clip(idx, 0, H - 1)
    ThT = np.where(mask, kh[:, :, idxc], 0.0)             # (L,D,128,128)
    TwT = np.where(mask, kw[:, :, idxc], 0.0)

    # D-skip diagonal tiles: dii[l,d] = D[l,d] * I  (bf16, for PE MMd)
    eye = np.eye(128, dtype=np.float64)
    dii = (Dskip[:, :, None, None] * eye[None, None]).astype(np.float32)

    flags = dict(
        use_ln_affine=not (np.all(ln_w == 1.0) and np.all(ln_b == 0.0)),
        use_b_out=not np.all(b_out2 == 0.0),
        n_layers=L,
        d_model=D,
        b_dec=float(b_dec[0]),
    )

    common = dict(
        wenc=W_enc.astype(np.float32).astype(bf16),                       # (2,D)
        benc_rep=np.tile(b_enc.astype(np.float32)[None, :], (128, 1)),    # (128,D) f32
        tht=ThT.astype(np.float32).astype(bf16),                          # (L,D,128,128)
        twt=TwT.astype(np.float32).astype(bf16),
        dii=dii.astype(bf16),                                             # (L,D,128,128)
        wdec_rep=np.tile(W_dec.astype(np.float32).reshape(1, D), (128, 1)).astype(bf16),
        ident=np.eye(128, dtype=np.float32).astype(bf16),
    )
    nk = max(1, D // 128)
    common["wout"] = np.ascontiguousarray(
        W_out2.reshape(L, nk, min(D, 128), 2 * D).astype(np.float32).astype(bf16)
    )
    if flags["use_ln_affine"]:
        common["lnw_rep"] = np.tile(ln_w.astype(np.float32)[:, None, :], (1, 128, 1)).astype(bf16)
        common["lnb_rep"] = np.tile(ln_b.astype(np.float32)[:, None, :], (1, 128, 1)).astype(bf16)
    if flags["use_b_out"]:
        common["bout_rep"] = np.tile(b_out2.astype(np.float32)[:, None, :], (1, 128, 1))

    per_batch = []
    for b in range(x.shape[0]):
        # xg[0, w*128+h] = x[b,h,w];  xg[1,...] = grid
        xb = x[b, :, :, 0].T.reshape(-1)     # (w,h) order
        gb = grid[b, :, :, 0].T.reshape(-1)
        xg = np.stack([xb, gb], axis=0).astype(np.float32).astype(bf16)
        per_batch.append(dict(xg=xg))
    return common, per_batch, flags


def build_program(flags, num_devices=8, gelu_fn=None):
    """Emit the bass program. Returns the compiled Bacc."""
    L = flags["n_layers"]
    D = flags["d_model"]
    DH = D // 2            # channels per half
    NK = max(1, D // 128)  # K tiles in W_out GEMM
    assert D % 2 == 0

    if gelu_fn is None:
        gelu_fn = AF.Gelu_apprx_tanh
    nc = bacc.Bacc("TRN2", target_bir_lowering=False, debug=False,
                   num_devices=num_devices)

    def din(name, shape, dt):
        return nc.dram_tensor(name, shape, dt, kind="ExternalInput").ap()

    xg = din("xg", [2, H * W], BF)
    wenc = din("wenc", [2, D], BF)
    benc_rep = din("benc_rep", [128, D], F32)
    tht = din("tht", [L, D, 128, 128], BF)
    twt = din("twt", [L, D, 128, 128], BF)
    dii = din("dii", [L, D, 128, 128], BF)
    wout = din("wout", [L, NK, min(D, 128), 2 * D], BF)
    wdec_rep = din("wdec_rep", [128, D], BF)
    ident = din("ident", [128, 128], BF)
    if flags["use_ln_affine"]:
        lnw_rep = din("lnw_rep", [L, 128, D], BF)
        lnb_rep = din("lnb_rep", [L, 128, D], BF)
    if flags["use_b_out"]:
        bout_rep = din("bout_rep", [L, 128, 2 * D], F32)
    out = nc.dram_tensor("out", [H, W], F32, kind="ExternalOutput").ap()

    from contextlib import ExitStack
    with tile.TileContext(nc) as tc, ExitStack() as ctx:
        state = ctx.enter_context(tc.tile_pool(name="state", bufs=1))
        consts = ctx.enter_context(tc.tile_pool(name="consts", bufs=1))
        wring = ctx.enter_context(tc.tile_pool(name="wring", bufs=12))
        lring = ctx.enter_context(tc.tile_pool(name="lring", bufs=2))
        atring = ctx.enter_context(tc.tile_pool(name="atring", bufs=3))
        cring = ctx.enter_context(tc.tile_pool(name="cring", bufs=3))
        gring = ctx.enter_context(tc.tile_pool(name="gring", bufs=2))
        sring = ctx.enter_context(tc.tile_pool(name="sring", bufs=3))
        stats = ctx.enter_context(tc.tile_pool(name="stats", bufs=1))
        ps = ctx.enter_context(tc.tile_pool(name="ps", bufs=8, space="PSUM"))

        h_sb = state.tile([128, W, D], BF, tag="h")
        Ysb = state.tile([128, DH, W], BF, tag="y")
        Xts = [state.tile([128, H * W], BF, tag=f"xt{k}", name=f"xt{k}")
               for k in range(NK)]

        wenc_t = consts.tile([2, D], BF)
        nc.sync.dma_start(wenc_t[:], wenc[:])
        benc_t = consts.tile([128, D], F32)
        nc.sync.dma_start(benc_t[:], benc_rep[:])
        ident_t = consts.tile([128, 128], BF)
        nc.sync.dma_start(ident_t[:], ident[:])
        wdec_t = consts.tile([128, D], BF)
        nc.sync.dma_start(wdec_t[:], wdec_rep[:])
        eps_t = consts.tile([128, 1], F32)
        nc.vector.memset(eps_t[:], 1e-5)

        # ---------------- encoder ----------------
        for w0 in range(W):
            xg_t = wring.tile([2, 128], BF, tag="xg")
            nc.sync.dma_start(xg_t[:], xg[:, w0 * 128:(w0 + 1) * 128])
            psE = ps.tile([128, 2 * D], F32, tag="ps")
            nc.tensor.matmul(psE[:, 0:D], xg_t[:], wenc_t[:],
                             start=True, stop=True)
            nc.vector.scalar_tensor_tensor(
                out=h_sb[:, w0, :], in0=psE[:, 0:D], scalar=1.0,
                in1=benc_t[:], op0=OP.mult, op1=OP.add)

        # ---------------- layers ----------------
        for l in range(L):
            wout_ts = []
            for k in range(NK):
                wt = lring.tile([min(D, 128), 2 * D], BF, tag="woutw")
                nc.sync.dma_start(wt[:], wout[l, k])
                wout_ts.append(wt)
            if flags["use_ln_affine"]:
                lnw_t = lring.tile([128, D], BF, tag="lnw")
                nc.sync.dma_start(lnw_t[:], lnw_rep[l])
                lnb_t = lring.tile([128, D], BF, tag="lnb")
                nc.sync.dma_start(lnb_t[:], lnb_rep[l])
            if flags["use_b_out"]:
                bout_t = lring.tile([128, 2 * D], F32, tag="bout")
                nc.sync.dma_start(bout_t[:], bout_rep[l])

            for hf in range(2):
                # ---- stage A: convolutions, 2 pairs (4 channels) at a time
                for dm in range(0, DH, 4):
                    d = hf * DH + dm
                    thts, twts, diis = [], [], []
                    for j in range(4):
                        tt_ = wring.tile([128, 128], BF, tag="tht")
                        nc.sync.dma_start(tt_[:], tht[l, d + j])
                        thts.append(tt_)
                        tw_ = wring.tile([128, 128], BF, tag="twt")
                        nc.sync.dma_start(tw_[:], twt[l, d + j])
                        twts.append(tw_)
                        di_ = wring.tile([128, 128], BF, tag="dii")
                        nc.sync.dma_start(di_[:], dii[l, d + j])
                        diis.append(di_)

                    slot1 = ps.tile([128, 4, 128], F32, tag="ps")
                    for j in range(4):
                        nc.tensor.matmul(slot1[:, j, :], h_sb[:, :, d + j],
                                         thts[j][:], start=True, stop=True)
                    At4 = atring.tile([128, 4, 128], BF, tag="at")
                    nc.scalar.copy(At4[:], slot1[:])

                    slot2 = ps.tile([128, 4, 128], F32, tag="ps")
                    for j in range(4):
                        nc.tensor.matmul(slot2[:, j, :], At4[:, j, :],
                                         twts[j][:], start=True, stop=False)
                        nc.tensor.matmul(slot2[:, j, :], diis[j][:],
                                         h_sb[:, :, d + j],
                                         start=False, stop=True)
                    # copy-out to Ysb (channel-major, contiguous dst)
                    nc.vector.tensor_copy(
                        Ysb[:, dm:dm + 4, :].rearrange("p a b -> p (a b)"),
                        slot2[:].rearrange("p a b -> p (a b)"))

                # ---- stage B: PE transpose + gelu into Xt ----
                for w0 in range(0, W, 4):
                    pst = ps.tile([128, 4, 128], BF, tag="ps")
                    for i in range(4):
                        nc.tensor.transpose(pst[:, i, :], Ysb[:, :, w0 + i],
                                            ident_t[:])
                    nc.scalar.activation(
                        Xts[hf][:, w0 * 128:(w0 + 4) * 128],
                        pst[:].rearrange("p a b -> p (a b)"), gelu_fn)

            # ---- stage C: W_out GEMM + GLU + residual + stats ----
            ssum = stats.tile([128, W], F32, tag="ssum")
            sqs = stats.tile([128, W], F32, tag="sqs")
            mu = stats.tile([128, W], F32, tag="mu")
            var = stats.tile([128, W], F32, tag="var")
            std = stats.tile([128, W], F32, tag="std")
            rr = stats.tile([128, W], F32, tag="rr")
            nmr = stats.tile([128, W], F32, tag="nmr")
            RB = 8  # residual batch (w0 per batched residual add)
            glu_big = None
            for wv in range(W):
                if wv % RB == 0:
                    glu_big = gring.tile([128, RB, D], BF, tag="glu")
                psW = ps.tile([128, 2 * D], F32, tag="ps")
                for k in range(NK):
                    nc.tensor.matmul(
                        psW[:],
                        Xts[k][:, wv * 128:(wv + 1) * 128],
                        wout_ts[k][:], start=(k == 0),
                        stop=(k == NK - 1))
                if flags["use_b_out"]:
                    nc.vector.tensor_tensor(psW[:], psW[:], bout_t[:],
                                            op=OP.add)
                # t1 = tanh(g/2); glu = (t1+1)*a'  (a' = 0.5*a baked in W_out)
                t1 = cring.tile([128, D], BF, tag="t1")
                nc.scalar.activation(t1[:], psW[:, D:2 * D], AF.Tanh,
                                     scale=0.5)
                g_sl = glu_big[:, wv % RB, :]
                if l > 0:
                    # sum_d h == 0 (LN output) => ssum accumulates on glu
                    nc.vector.scalar_tensor_tensor(
                        out=g_sl, in0=t1[:], scalar=1.0, in1=psW[:, 0:D],
                        op0=OP.add, op1=OP.mult,
                        accum_out=ssum[:, wv:wv + 1])
                else:
                    nc.vector.scalar_tensor_tensor(
                        out=g_sl, in0=t1[:], scalar=1.0, in1=psW[:, 0:D],
                        op0=OP.add, op1=OP.mult)
                if wv % RB == RB - 1:
                    b0 = wv - RB + 1
                    if l > 0:
                        # batched residual: h += glu
                        nc.vector.tensor_tensor(
                            h_sb[:, b0:wv + 1, :].rearrange("p a b -> p (a b)"),
                            glu_big[:].rearrange("p a b -> p (a b)"),
                            h_sb[:, b0:wv + 1, :].rearrange("p a b -> p (a b)"),
                            op=OP.add)
                    else:
                        for wx in range(b0, wv + 1):
                            nc.vector.scalar_tensor_tensor(
                                out=h_sb[:, wx, :], in0=glu_big[:, wx % RB, :],
                                scalar=1.0, in1=h_sb[:, wx, :],
                                op0=OP.mult, op1=OP.add,
                                accum_out=ssum[:, wx:wx + 1])
                    # sumsq per w0 (alternate DVE/ACT)
                    for wx in range(b0, wv + 1):
                        scr = cring.tile([128, D], BF, tag="scr")
                        if wx % 2 == 0:
                            nc.vector.scalar_tensor_tensor(
                                out=scr[:], in0=h_sb[:, wx, :], scalar=1.0,
                                in1=h_sb[:, wx, :], op0=OP.mult, op1=OP.mult,
                                accum_out=sqs[:, wx:wx + 1])
                        else:
                            nc.scalar.activation(
                                scr[:], h_sb[:, wx, :], AF.Square,
                                accum_out=sqs[:, wx:wx + 1])

            # ---- batched stats ----
            nc.vector.tensor_scalar(out=mu[:], in0=ssum[:],
                                    scalar1=1.0 / D, scalar2=None,
                                    op0=OP.mult)
            nc.vector.tensor_tensor(var[:], mu[:], mu[:], op=OP.mult)
            nc.vector.scalar_tensor_tensor(
                out=var[:], in0=sqs[:], scalar=1.0 / D,
                in1=var[:], op0=OP.mult, op1=OP.subtract)
            nc.scalar.activation(std[:], var[:], AF.Sqrt, bias=eps_t[:, 0:1])
            nc.vector.reciprocal(rr[:], std[:])
            nc.vector.scalar_tensor_tensor(
                out=nmr[:], in0=mu[:], scalar=-1.0,
                in1=rr[:], op0=OP.mult, op1=OP.mult)

            # ---- normalize pass (alternate DVE/ACT) ----
            for w0 in range(W):
                if w0 % 2 == 0:
                    nc.vector.tensor_scalar(
                        out=h_sb[:, w0, :], in0=h_sb[:, w0, :],
                        scalar1=rr[:, w0:w0 + 1], scalar2=nmr[:, w0:w0 + 1],
                        op0=OP.mult, op1=OP.add)
                else:
                    nc.scalar.activation(
                        h_sb[:, w0, :], h_sb[:, w0, :], AF.Identity,
                        bias=nmr[:, w0:w0 + 1], scale=rr[:, w0:w0 + 1])
                if flags["use_ln_affine"]:
                    nc.vector.tensor_tensor(
                        h_sb[:, w0, :], h_sb[:, w0, :], lnw_t[:],
                        op=OP.mult)
                    nc.vector.tensor_tensor(
                        h_sb[:, w0, :], h_sb[:, w0, :], lnb_t[:],
                        op=OP.add)

        # ---------------- decoder ----------------
        dec_sb = consts.tile([128, W], F32)
        for w0 in range(W):
            scr = cring.tile([128, D], BF, tag="scr")
            nc.vector.scalar_tensor_tensor(
                out=scr[:], in0=h_sb[:, w0, :], scalar=1.0, in1=wdec_t[:],
                op0=OP.mult, op1=OP.mult, accum_out=dec_sb[:, w0:w0 + 1])
        if flags["b_dec"] != 0.0:
            nc.vector.tensor_scalar(out=dec_sb[:], in0=dec_sb[:],
                                    scalar1=float(flags["b_dec"]), scalar2=None,
                                    op0=OP.add)
        nc.sync.dma_start(out[:], dec_sb[:])

    nc.compile()
    return nc


# ---------------------------------------------------------------------------
# Self-contained entry point: full inputs in, full output out.
# Shards batch-parallel across 8 NeuronCores (cores 4..7 duplicate work).
# ---------------------------------------------------------------------------

_PROGRAM_CACHE = {}


def _get_program(flags):
    key = (flags["n_layers"], flags["d_model"], flags["use_ln_affine"],
           flags["use_b_out"], flags["b_dec"])
    if key not in _PROGRAM_CACHE:
        _PROGRAM_CACHE[key] = build_program(flags, num_devices=8)
    return _PROGRAM_CACHE[key]


def kernel(**inputs):
    import os
    from concourse.bass_utils import run_bass_kernel_spmd

    common, per_batch, flags = host_prep(inputs)
    nc = _get_program(flags)

    B = len(per_batch)
    in_maps = []
    for c in range(8):
        m = dict(common)
        m.update(per_batch[c % B])
        in_maps.append(m)

    trace = bool(os.environ.get("S4ND_TRACE"))
    res = run_bass_kernel_spmd(nc, in_maps, core_ids=list(range(8)), trace=trace)
    if trace and res.exec_time_ns is not None:
        print(f"HW exec time: {res.exec_time_ns} ns")
        kernel.last_exec_time_ns = res.exec_time_ns
        kernel.last_results = res

    out = np.stack([res.results[b]["out"] for b in range(B)], axis=0)[..., None]
    return out.astype(np.float32)


# revision 12
# speedup vs baseline: 1.9835x; 1.2893x over previous
"""S4ND Darcy-flow Bass kernel v3: builder + host-side preparation.

Design (per core = one batch element, batch-parallel over 4 cores, cores
4..7 duplicate work and are ignored at gather time):

  state h_sb: SBUF bf16 [128p=h, (w, d)], d innermost.
  Per layer, per half hf (128 channels each):
    stage A (conv), per 2-pair group (4 channels), one 8-bank psum pool:
      slot1 <- MM1 x4: A^T[w, h'] = U_d^T @ ThT_d
      copy1 (ACT, FD=512): slot1 -> At4 bf16
      slot2 <- MM2 x4 + MMd x4 (accumulate):
               y[h', w'] = At_d @ TwT_d + (D_d I) @ U_d   (D-skip on PE,
               dI tiles shipped from host)
      Ycopy (DVE, FD=512): slot2 -> Ysb[:, w, 4ch] (w-major [128, W, DH])
    stage B, per 4-w0 quad:
      DMA transpose x4: Ysb[:, w0+i, :] -> stg [dm, 4, h]  (DMA xbar engine)
      gelu (ACT, FD=512): stg -> Xt[hf]
  stage C, per w0 (8-deep psum pipeline):
    W_out GEMM x2 (k chunks) into psW [128, 512]
    tanh (ACT): t1 = tanh(0.5*g)         [sigmoid via tanh: same act table]
    glu (DVE stt + accum): glu = (t1+1)*a', accum ssum  (for layers >= 1,
        sum_d h == 0 exactly since h is LayerNorm output, so sum_d p =
        sum_d glu; layer 0 accumulates on the residual op instead)
    sumsq (DVE/ACT alternating, + accum sqs)
  residual (DVE tt, batched FD=2048 per 8 w0): h_sb += glu_big
  stats (batched, per layer): mu/var (DVE), std=sqrt(var+eps) (ACT, one
    table switch), rr=1/std (DVE recip), nmr=-mu*rr (DVE)
  normalize, per w0 (DVE/ACT alternating, per-partition AP scalars):
    h_sb = p*rr + nmr
  Decoder: DVE stt dot-products per w slice -> out (h, w) f32.

Host precomputes (numpy, float64): S4D kernels kh/kw, transposed Toeplitz
matrices ThT/TwT, D*I diagonal tiles, replicated small tensors, xg packing.
W_out a-half is pre-scaled by 0.5 for the tanh-based GLU.
"""

import numpy as np
import ml_dtypes

import concourse.bacc as bacc
import concourse.mybir as mybir
import concourse.tile as tile

bf16 = ml_dtypes.bfloat16
AF = mybir.ActivationFunctionType
OP = mybir.AluOpType
F32 = mybir.dt.float32
BF = mybir.dt.bfloat16

H = 128
W = 128


def host_prep(inputs, n_layers=None, d_model=None):
    """Compute device-side constant tensors from the full model inputs."""
    log_dt = np.asarray(inputs["log_dt"], np.float64)     # (L,2,d)
    logA_re = np.asarray(inputs["logA_re"], np.float64)   # (L,2,d,N)
    A_im = np.asarray(inputs["A_im"], np.float64)
    C_re = np.asarray(inputs["C_re"], np.float64)
    C_im = np.asarray(inputs["C_im"], np.float64)
    Dskip = np.asarray(inputs["Dskip"], np.float64)       # (L,d)
    W_out = np.asarray(inputs["W_out"], np.float64)       # (L,d,2d)
    b_out = np.asarray(inputs["b_out"], np.float64)       # (L,2d)
    ln_w = np.asarray(inputs["ln_w"], np.float64)         # (L,d)
    ln_b = np.asarray(inputs["ln_b"], np.float64)
    W_enc = np.asarray(inputs["W_enc"], np.float64)       # (2,d)
    b_enc = np.asarray(inputs["b_enc"], np.float64)       # (d,)
    W_dec = np.asarray(inputs["W_dec"], np.float64)       # (d,1)
    b_dec = np.asarray(inputs["b_dec"], np.float64)       # (1,)
    x = np.asarray(inputs["x"], np.float32)               # (B,H,W,1)
    grid = np.asarray(inputs["grid"], np.float32)

    L = log_dt.shape[0] if n_layers is None else n_layers
    D = log_dt.shape[2] if d_model is None else d_model
    log_dt = log_dt[:L, :, :D]
    logA_re = logA_re[:L, :, :D]
    A_im = A_im[:L, :, :D]
    C_re = C_re[:L, :, :D]
    C_im = C_im[:L, :, :D]
    Dskip = Dskip[:L, :D]
    d_full = W_out.shape[1]
    Wa = W_out[:L, :D, :D] * 0.5          # pre-scale a-half for tanh GLU
    Wg = W_out[:L, :D, d_full:d_full + D]
    W_out2 = np.concatenate([Wa, Wg], axis=2)             # (L, D, 2D)
    b_out2 = np.concatenate([b_out[:L, :D] * 0.5,
                             b_out[:L, d_full:d_full + D]], axis=1)
    ln_w = ln_w[:L, :D]
    ln_b = ln_b[:L, :D]
    W_enc = W_enc[:, :D]
    b_enc = b_enc[:D]
    W_dec = W_dec[:D]

    # ---- S4D kernels ----
    dt = np.exp(log_dt)[..., None]                        # (L,2,D,1)
    A = -np.exp(logA_re) + 1j * A_im                      # (L,2,D,N)
    C = C_re + 1j * C_im
    dtA = dt * A
    CB = C * (np.exp(dtA) - 1.0) / A
    t = np.arange(H, dtype=np.float64)
    pows = np.exp(dtA[..., None] * t)                     # (L,2,D,N,H)
    K = 2.0 * np.real(np.einsum("lxdn,lxdnt->lxdt", CB, pows))  # (L,2,D,H)
    kh = K[:, 0]                                          # (L,D,H)
    kw = K[:, 1]                                          # (L,D,W)

    # transposed lower-triangular Toeplitz: ThT[l,d,i,p] = kh[l,d,p-i], p>=i
    idx = np.arange(H)[None, :] - np.arange(H)[:, None]   # (i,p) = p-i
    mask = idx >= 0
    idxc = np.clip(idx, 0, H - 1)
    ThT = np.where(mask, kh[:, :, idxc], 0.0)             # (L,D,128,128)
    TwT = np.where(mask, kw[:, :, idxc], 0.0)

    flags = dict(
        use_ln_affine=not (np.all(ln_w == 1.0) and np.all(ln_b == 0.0)),
        use_b_out=not np.all(b_out2 == 0.0),
        n_layers=L,
        d_model=D,
        b_dec=float(b_dec[0]),
    )

    common = dict(
        wenc=W_enc.astype(np.float32).astype(bf16),                       # (2,D)
        benc_rep=np.tile(b_enc.astype(np.float32)[None, :], (128, 1)),    # (128,D) f32
        tht=ThT.astype(np.float32).astype(bf16),                          # (L,D,128,128)
        twt=TwT.astype(np.float32).astype(bf16),
        drep=np.tile(Dskip.astype(np.float32)[:, None, :], (1, 128, 1)),  # (L,128,D) f32
        wdec_rep=np.tile(W_dec.astype(np.float32).reshape(1, D), (128, 1)).astype(bf16),
        ident=np.eye(128, dtype=np.float32).astype(bf16),
    )
    nk = max(1, D // 128)
    common["wout"] = np.ascontiguousarray(
        W_out2.reshape(L, nk, min(D, 128), 2 * D).astype(np.float32).astype(bf16)
    )
    if flags["use_ln_affine"]:
        common["lnw_rep"] = np.tile(ln_w.astype(np.float32)[:, None, :], (1, 128, 1)).astype(bf16)
        common["lnb_rep"] = np.tile(ln_b.astype(np.float32)[:, None, :], (1, 128, 1)).astype(bf16)
    if flags["use_b_out"]:
        common["bout_rep"] = np.tile(b_out2.astype(np.float32)[:, None, :], (1, 128, 1))

    per_batch = []
    for b in range(x.shape[0]):
        # xg[0, w*128+h] = x[b,h,w];  xg[1,...] = grid
        xb = x[b, :, :, 0].T.reshape(-1)     # (w,h) order
        gb = grid[b, :, :, 0].T.reshape(-1)
        xg = np.stack([xb, gb], axis=0).astype(np.float32).astype(bf16)
        per_batch.append(dict(xg=xg))
    return common, per_batch, flags


def build_program(flags, num_devices=8, gelu_fn=None):
    """Emit the bass program. Returns the compiled Bacc."""
    L = flags["n_layers"]
    D = flags["d_model"]
    DH = D // 2            # channels per half
    NK = max(1, D // 128)  # K tiles in W_out GEMM
    assert D % 2 == 0

    if gelu_fn is None:
        gelu_fn = AF.Gelu_apprx_tanh
    nc = bacc.Bacc("TRN2", target_bir_lowering=False, debug=False,
                   num_devices=num_devices)

    def din(name, shape, dt):
        return nc.dram_tensor(name, shape, dt, kind="ExternalInput").ap()

    xg = din("xg", [2, H * W], BF)
    wenc = din("wenc", [2, D], BF)
    benc_rep = din("benc_rep", [128, D], F32)
    tht = din("tht", [L, D, 128, 128], BF)
    twt = din("twt", [L, D, 128, 128], BF)
    drep = din("drep", [L, 128, D], F32)
    wout = din("wout", [L, NK, min(D, 128), 2 * D], BF)
    wdec_rep = din("wdec_rep", [128, D], BF)
    ident = din("ident", [128, 128], BF)
    if flags["use_ln_affine"]:
        lnw_rep = din("lnw_rep", [L, 128, D], BF)
        lnb_rep = din("lnb_rep", [L, 128, D], BF)
    if flags["use_b_out"]:
        bout_rep = din("bout_rep", [L, 128, 2 * D], F32)
    out = nc.dram_tensor("out", [H, W], F32, kind="ExternalOutput").ap()

    from contextlib import ExitStack
    with tile.TileContext(nc) as tc, ExitStack() as ctx:
        state = ctx.enter_context(tc.tile_pool(name="state", bufs=1))
        consts = ctx.enter_context(tc.tile_pool(name="consts", bufs=1))
        wring = ctx.enter_context(tc.tile_pool(name="wring", bufs=12))
        lring = ctx.enter_context(tc.tile_pool(name="lring", bufs=2))
        atring = ctx.enter_context(tc.tile_pool(name="atring", bufs=3))
        cring = ctx.enter_context(tc.tile_pool(name="cring", bufs=3))
        gring = ctx.enter_context(tc.tile_pool(name="gring", bufs=2))
        sring = ctx.enter_context(tc.tile_pool(name="sring", bufs=3))
        stats = ctx.enter_context(tc.tile_pool(name="stats", bufs=1))
        ps = ctx.enter_context(tc.tile_pool(name="ps", bufs=8, space="PSUM"))

        h_sb = state.tile([128, W, D], BF, tag="h")
        Ysb = state.tile([128, DH, W], BF, tag="y")
        Xts = [state.tile([128, H * W], BF, tag=f"xt{k}", name=f"xt{k}")
               for k in range(NK)]

        wenc_t = consts.tile([2, D], BF)
        nc.sync.dma_start(wenc_t[:], wenc[:])
        benc_t = consts.tile([128, D], F32)
        nc.sync.dma_start(benc_t[:], benc_rep[:])
        ident_t = consts.tile([128, 128], BF)
        nc.sync.dma_start(ident_t[:], ident[:])
        wdec_t = consts.tile([128, D], BF)
        nc.sync.dma_start(wdec_t[:], wdec_rep[:])
        eps_t = consts.tile([128, 1], F32)
        nc.vector.memset(eps_t[:], 1e-5)

        # ---------------- encoder ----------------
        for w0 in range(W):
            xg_t = wring.tile([2, 128], BF, tag="xg")
            nc.sync.dma_start(xg_t[:], xg[:, w0 * 128:(w0 + 1) * 128])
            psE = ps.tile([128, 2 * D], F32, tag="ps")
            nc.tensor.matmul(psE[:, 0:D], xg_t[:], wenc_t[:],
                             start=True, stop=True)
            nc.vector.scalar_tensor_tensor(
                out=h_sb[:, w0, :], in0=psE[:, 0:D], scalar=1.0,
                in1=benc_t[:], op0=OP.mult, op1=OP.add)

        # ---------------- layers ----------------
        for l in range(L):
            wout_ts = []
            for k in range(NK):
                wt = lring.tile([min(D, 128), 2 * D], BF, tag="woutw")
                nc.sync.dma_start(wt[:], wout[l, k])
                wout_ts.append(wt)
            drep_t = lring.tile([128, D], F32, tag="drep")
            nc.sync.dma_start(drep_t[:], drep[l])
            if flags["use_ln_affine"]:
                lnw_t = lring.tile([128, D], BF, tag="lnw")
                nc.sync.dma_start(lnw_t[:], lnw_rep[l])
                lnb_t = lring.tile([128, D], BF, tag="lnb")
                nc.sync.dma_start(lnb_t[:], lnb_rep[l])
            if flags["use_b_out"]:
                bout_t = lring.tile([128, 2 * D], F32, tag="bout")
                nc.sync.dma_start(bout_t[:], bout_rep[l])

            for hf in range(2):
                # ---- stage A: convolutions, 2 pairs (4 channels) at a time
                for dm in range(0, DH, 4):
                    d = hf * DH + dm
                    thts, twts = [], []
                    for j in range(4):
                        tt_ = wring.tile([128, 128], BF, tag="tht")
                        nc.sync.dma_start(tt_[:], tht[l, d + j])
                        thts.append(tt_)
                        tw_ = wring.tile([128, 128], BF, tag="twt")
                        nc.sync.dma_start(tw_[:], twt[l, d + j])
                        twts.append(tw_)

                    slot1 = ps.tile([128, 4, 128], F32, tag="ps")
                    for j in range(4):
                        nc.tensor.matmul(slot1[:, j, :], h_sb[:, :, d + j],
                                         thts[j][:], start=True, stop=True)
                    At4 = atring.tile([128, 4, 128], BF, tag="at")
                    nc.scalar.copy(At4[:], slot1[:])

                    slot2 = ps.tile([128, 4, 128], F32, tag="ps")
                    for j in range(4):
                        nc.tensor.matmul(slot2[:, j, :], At4[:, j, :],
                                         twts[j][:], start=True, stop=True)
                    # D-skip fused with copy-out to Ysb (channel-major)
                    for j in range(4):
                        nc.vector.scalar_tensor_tensor(
                            out=Ysb[:, dm + j, :], in0=h_sb[:, :, d + j],
                            scalar=drep_t[:, d + j:d + j + 1],
                            in1=slot2[:, j, :], op0=OP.mult, op1=OP.add)

                # ---- stage B: PE transpose + gelu into Xt ----
                for w0 in range(0, W, 4):
                    pst = ps.tile([128, 4, 128], BF, tag="ps")
                    for i in range(4):
                        nc.tensor.transpose(pst[:, i, :], Ysb[:, :, w0 + i],
                                            ident_t[:])
                    nc.scalar.activation(
                        Xts[hf][:, w0 * 128:(w0 + 4) * 128],
                        pst[:].rearrange("p a b -> p (a b)"), gelu_fn)

            # ---- stage C: W_out GEMM + GLU + residual + stats ----
            ssum = stats.tile([128, W], F32, tag="ssum")
            sqs = stats.tile([128, W], F32, tag="sqs")
            mu = stats.tile([128, W], F32, tag="mu")
            var = stats.tile([128, W], F32, tag="var")
            std = stats.tile([128, W], F32, tag="std")
            rr = stats.tile([128, W], F32, tag="rr")
            nmr = stats.tile([128, W], F32, tag="nmr")
            RB = 8  # residual batch (w0 per batched residual add)
            glu_big = None
            for wv in range(W):
                if wv % RB == 0:
                    glu_big = gring.tile([128, RB, D], BF, tag="glu")
                psW = ps.tile([128, 2 * D], F32, tag="ps")
                for k in range(NK):
                    nc.tensor.matmul(
                        psW[:],
                        Xts[k][:, wv * 128:(wv + 1) * 128],
                        wout_ts[k][:], start=(k == 0),
                        stop=(k == NK - 1))
                if flags["use_b_out"]:
                    nc.vector.tensor_tensor(psW[:], psW[:], bout_t[:],
                                            op=OP.add)
                # t1 = tanh(g/2); glu = (t1+1)*a'  (a' = 0.5*a baked in W_out)
                t1 = cring.tile([128, D], BF, tag="t1")
                nc.scalar.activation(t1[:], psW[:, D:2 * D], AF.Tanh,
                                     scale=0.5)
                g_sl = glu_big[:, wv % RB, :]
                if l > 0:
                    # sum_d h == 0 (LN output) => ssum accumulates on glu
                    nc.vector.scalar_tensor_tensor(
                        out=g_sl, in0=t1[:], scalar=1.0, in1=psW[:, 0:D],
                        op0=OP.add, op1=OP.mult,
                        accum_out=ssum[:, wv:wv + 1])
                else:
                    nc.vector.scalar_tensor_tensor(
                        out=g_sl, in0=t1[:], scalar=1.0, in1=psW[:, 0:D],
                        op0=OP.add, op1=OP.mult)
                if wv % RB == RB - 1:
                    b0 = wv - RB + 1
                    if l > 0:
                        # batched residual: h += glu
                        nc.vector.tensor_tensor(
                            h_sb[:, b0:wv + 1, :].rearrange("p a b -> p (a b)"),
                            glu_big[:].rearrange("p a b -> p (a b)"),
                            h_sb[:, b0:wv + 1, :].rearrange("p a b -> p (a b)"),
                            op=OP.add)
                    else:
                        for wx in range(b0, wv + 1):
                            nc.vector.scalar_tensor_tensor(
                                out=h_sb[:, wx, :], in0=glu_big[:, wx % RB, :],
                                scalar=1.0, in1=h_sb[:, wx, :],
                                op0=OP.mult, op1=OP.add,
                                accum_out=ssum[:, wx:wx + 1])
                    # sumsq per w0 (alternate DVE/ACT)
                    for wx in range(b0, wv + 1):
                        scr = cring.tile([128, D], BF, tag="scr")
                        if wx % 2 == 0:
                            nc.vector.scalar_tensor_tensor(
                                out=scr[:], in0=h_sb[:, wx, :], scalar=1.0,
                                in1=h_sb[:, wx, :], op0=OP.mult, op1=OP.mult,
                                accum_out=sqs[:, wx:wx + 1])
                        else:
                            nc.scalar.activation(
                                scr[:], h_sb[:, wx, :], AF.Square,
                                accum_out=sqs[:, wx:wx + 1])

            # ---- batched stats ----
            nc.vector.tensor_scalar(out=mu[:], in0=ssum[:],
                                    scalar1=1.0 / D, scalar2=None,
                                    op0=OP.mult)
            nc.vector.tensor_tensor(var[:], mu[:], mu[:], op=OP.mult)
            nc.vector.scalar_tensor_tensor(
                out=var[:], in0=sqs[:], scalar=1.0 / D,
                in1=var[:], op0=OP.mult, op1=OP.subtract)
            nc.scalar.activation(std[:], var[:], AF.Sqrt, bias=eps_t[:, 0:1])
            nc.vector.reciprocal(rr[:], std[:])
            nc.vector.scalar_tensor_tensor(
                out=nmr[:], in0=mu[:], scalar=-1.0,
                in1=rr[:], op0=OP.mult, op1=OP.mult)

            # ---- normalize pass (alternate DVE/ACT) ----
            for w0 in range(W):
                if w0 % 8 == 4:
                    # keep-warm: idle PE re-throttles to 1.2 GHz after ~3.4us;
                    # a cheap transpose per 8 w0 keeps the HAM busy window hot
                    warm = ps.tile([128, 128], BF, tag="ps")
                    nc.tensor.transpose(warm[:], ident_t[:], ident_t[:])
                if w0 % 2 == 0:
                    nc.vector.tensor_scalar(
                        out=h_sb[:, w0, :], in0=h_sb[:, w0, :],
                        scalar1=rr[:, w0:w0 + 1], scalar2=nmr[:, w0:w0 + 1],
                        op0=OP.mult, op1=OP.add)
                else:
                    nc.scalar.activation(
                        h_sb[:, w0, :], h_sb[:, w0, :], AF.Identity,
                        bias=nmr[:, w0:w0 + 1], scale=rr[:, w0:w0 + 1])
                if flags["use_ln_affine"]:
                    nc.vector.tensor_tensor(
                        h_sb[:, w0, :], h_sb[:, w0, :], lnw_t[:],
                        op=OP.mult)
                    nc.vector.tensor_tensor(
                        h_sb[:, w0, :], h_sb[:, w0, :], lnb_t[:],
                        op=OP.add)

        # ---------------- decoder ----------------
        dec_sb = consts.tile([128, W], F32)
        for w0 in range(W):
            scr = cring.tile([128, D], BF, tag="scr")
            nc.vector.scalar_tensor_tensor(
                out=scr[:], in0=h_sb[:, w0, :], scalar=1.0, in1=wdec_t[:],
                op0=OP.mult, op1=OP.mult, accum_out=dec_sb[:, w0:w0 + 1])
        if flags["b_dec"] != 0.0:
            nc.vector.tensor_scalar(out=dec_sb[:], in0=dec_sb[:],
                                    scalar1=float(flags["b_dec"]), scalar2=None,
                                    op0=OP.add)
        nc.sync.dma_start(out[:], dec_sb[:])

    nc.compile()
    return nc


# ---------------------------------------------------------------------------
# Self-contained entry point: full inputs in, full output out.
# Shards batch-parallel across 8 NeuronCores (cores 4..7 duplicate work).
# ---------------------------------------------------------------------------

_PROGRAM_CACHE = {}


def _get_program(flags):
    key = (flags["n_layers"], flags["d_model"], flags["use_ln_affine"],
           flags["use_b_out"], flags["b_dec"])
    if key not in _PROGRAM_CACHE:
        _PROGRAM_CACHE[key] = build_program(flags, num_devices=8)
    return _PROGRAM_CACHE[key]


def kernel(**inputs):
    import os
    from concourse.bass_utils import run_bass_kernel_spmd

    common, per_batch, flags = host_prep(inputs)
    nc = _get_program(flags)

    B = len(per_batch)
    in_maps = []
    for c in range(8):
        m = dict(common)
        m.update(per_batch[c % B])
        in_maps.append(m)

    trace = bool(os.environ.get("S4ND_TRACE"))
    res = run_bass_kernel_spmd(nc, in_maps, core_ids=list(range(8)), trace=trace)
    if trace and res.exec_time_ns is not None:
        print(f"HW exec time: {res.exec_time_ns} ns")
        kernel.last_exec_time_ns = res.exec_time_ns
        kernel.last_results = res

    out = np.stack([res.results[b]["out"] for b in range(B)], axis=0)[..., None]
    return out.astype(np.float32)


# revision 15
# speedup vs baseline: 2.1018x; 1.0596x over previous
"""S4ND Darcy-flow Bass kernel v3: builder + host-side preparation.

Design (per core = one batch element, batch-parallel over 4 cores, cores
4..7 duplicate work and are ignored at gather time):

  state h_sb: SBUF bf16 [128p=h, (w, d)], d innermost.
  Per layer, per half hf (128 channels each):
    stage A (conv), per 2-pair group (4 channels), one 8-bank psum pool:
      slot1 <- MM1 x4: A^T[w, h'] = U_d^T @ ThT_d
      copy1 (ACT, FD=512): slot1 -> At4 bf16
      slot2 <- MM2 x4 + MMd x4 (accumulate):
               y[h', w'] = At_d @ TwT_d + (D_d I) @ U_d   (D-skip on PE,
               dI tiles shipped from host)
      Ycopy (DVE, FD=512): slot2 -> Ysb[:, w, 4ch] (w-major [128, W, DH])
    stage B, per 4-w0 quad:
      DMA transpose x4: Ysb[:, w0+i, :] -> stg [dm, 4, h]  (DMA xbar engine)
      gelu (ACT, FD=512): stg -> Xt[hf]
  stage C, per w0 (8-deep psum pipeline):
    W_out GEMM x2 (k chunks) into psW [128, 512]
    tanh (ACT): t1 = tanh(0.5*g)         [sigmoid via tanh: same act table]
    glu (DVE stt + accum): glu = (t1+1)*a', accum ssum  (for layers >= 1,
        sum_d h == 0 exactly since h is LayerNorm output, so sum_d p =
        sum_d glu; layer 0 accumulates on the residual op instead)
    sumsq (DVE/ACT alternating, + accum sqs)
  residual (DVE tt, batched FD=2048 per 8 w0): h_sb += glu_big
  stats (batched, per layer): mu/var (DVE), std=sqrt(var+eps) (ACT, one
    table switch), rr=1/std (DVE recip), nmr=-mu*rr (DVE)
  normalize, per w0 (DVE/ACT alternating, per-partition AP scalars):
    h_sb = p*rr + nmr
  Decoder: DVE stt dot-products per w slice -> out (h, w) f32.

Host precomputes (numpy, float64): S4D kernels kh/kw, transposed Toeplitz
matrices ThT/TwT, D*I diagonal tiles, replicated small tensors, xg packing.
W_out a-half is pre-scaled by 0.5 for the tanh-based GLU.
"""

import numpy as np
import ml_dtypes

import concourse.bacc as bacc
import concourse.mybir as mybir
import concourse.tile as tile

bf16 = ml_dtypes.bfloat16
AF = mybir.ActivationFunctionType
OP = mybir.AluOpType
F32 = mybir.dt.float32
BF = mybir.dt.bfloat16

H = 128
W = 128


def host_prep(inputs, n_layers=None, d_model=None):
    """Compute device-side constant tensors from the full model inputs."""
    log_dt = np.asarray(inputs["log_dt"], np.float64)     # (L,2,d)
    logA_re = np.asarray(inputs["logA_re"], np.float64)   # (L,2,d,N)
    A_im = np.asarray(inputs["A_im"], np.float64)
    C_re = np.asarray(inputs["C_re"], np.float64)
    C_im = np.asarray(inputs["C_im"], np.float64)
    Dskip = np.asarray(inputs["Dskip"], np.float64)       # (L,d)
    W_out = np.asarray(inputs["W_out"], np.float64)       # (L,d,2d)
    b_out = np.asarray(inputs["b_out"], np.float64)       # (L,2d)
    ln_w = np.asarray(inputs["ln_w"], np.float64)         # (L,d)
    ln_b = np.asarray(inputs["ln_b"], np.float64)
    W_enc = np.asarray(inputs["W_enc"], np.float64)       # (2,d)
    b_enc = np.asarray(inputs["b_enc"], np.float64)       # (d,)
    W_dec = np.asarray(inputs["W_dec"], np.float64)       # (d,1)
    b_dec = np.asarray(inputs["b_dec"], np.float64)       # (1,)
    x = np.asarray(inputs["x"], np.float32)               # (B,H,W,1)
    grid = np.asarray(inputs["grid"], np.float32)

    L = log_dt.shape[0] if n_layers is None else n_layers
    D = log_dt.shape[2] if d_model is None else d_model
    log_dt = log_dt[:L, :, :D]
    logA_re = logA_re[:L, :, :D]
    A_im = A_im[:L, :, :D]
    C_re = C_re[:L, :, :D]
    C_im = C_im[:L, :, :D]
    Dskip = Dskip[:L, :D]
    d_full = W_out.shape[1]
    Wa = W_out[:L, :D, :D] * 0.5          # pre-scale a-half for tanh GLU
    Wg = W_out[:L, :D, d_full:d_full + D]
    W_out2 = np.concatenate([Wa, Wg], axis=2)             # (L, D, 2D)
    b_out2 = np.concatenate([b_out[:L, :D] * 0.5,
                             b_out[:L, d_full:d_full + D]], axis=1)
    ln_w = ln_w[:L, :D]
    ln_b = ln_b[:L, :D]
    W_enc = W_enc[:, :D]
    b_enc = b_enc[:D]
    W_dec = W_dec[:D]

    # ---- S4D kernels ----
    dt = np.exp(log_dt)[..., None]                        # (L,2,D,1)
    A = -np.exp(logA_re) + 1j * A_im                      # (L,2,D,N)
    C = C_re + 1j * C_im
    dtA = dt * A
    CB = C * (np.exp(dtA) - 1.0) / A
    t = np.arange(H, dtype=np.float64)
    pows = np.exp(dtA[..., None] * t)                     # (L,2,D,N,H)
    K = 2.0 * np.real(np.einsum("lxdn,lxdnt->lxdt", CB, pows))  # (L,2,D,H)
    kh = K[:, 0]                                          # (L,D,H)
    kw = K[:, 1]                                          # (L,D,W)

    # transposed lower-triangular Toeplitz: ThT[l,d,i,p] = kh[l,d,p-i], p>=i
    idx = np.arange(H)[None, :] - np.arange(H)[:, None]   # (i,p) = p-i
    mask = idx >= 0
    idxc = np.clip(idx, 0, H - 1)
    ThT = np.where(mask, kh[:, :, idxc], 0.0)             # (L,D,128,128)
    TwT = np.where(mask, kw[:, :, idxc], 0.0)

    flags = dict(
        use_ln_affine=not (np.all(ln_w == 1.0) and np.all(ln_b == 0.0)),
        use_b_out=not np.all(b_out2 == 0.0),
        n_layers=L,
        d_model=D,
        b_dec=float(b_dec[0]),
    )

    common = dict(
        wenc=W_enc.astype(np.float32).astype(bf16),                       # (2,D)
        benc_rep=np.tile(b_enc.astype(np.float32)[None, :], (128, 1)),    # (128,D) f32
        tht=ThT.astype(np.float32).astype(bf16),                          # (L,D,128,128)
        twt=TwT.astype(np.float32).astype(bf16),
        drep=np.tile(Dskip.astype(np.float32)[:, None, :], (1, 128, 1)),  # (L,128,D) f32
        wdec_rep=np.tile(W_dec.astype(np.float32).reshape(1, D), (128, 1)).astype(bf16),
        swd_rep=np.full((128, 1), float(np.sum(W_dec)), np.float32),
        ident=np.eye(128, dtype=np.float32).astype(bf16),
    )
    nk = max(1, D // 128)
    common["wout"] = np.ascontiguousarray(
        W_out2.reshape(L, nk, min(D, 128), 2 * D).astype(np.float32).astype(bf16)
    )
    if flags["use_ln_affine"]:
        common["lnw_rep"] = np.tile(ln_w.astype(np.float32)[:, None, :], (1, 128, 1)).astype(bf16)
        common["lnb_rep"] = np.tile(ln_b.astype(np.float32)[:, None, :], (1, 128, 1)).astype(bf16)
    if flags["use_b_out"]:
        common["bout_rep"] = np.tile(b_out2.astype(np.float32)[:, None, :], (1, 128, 1))

    per_batch = []
    for b in range(x.shape[0]):
        # xg[0, w*128+h] = x[b,h,w];  xg[1,...] = grid
        xb = x[b, :, :, 0].T.reshape(-1)     # (w,h) order
        gb = grid[b, :, :, 0].T.reshape(-1)
        xg = np.stack([xb, gb], axis=0).astype(np.float32).astype(bf16)
        per_batch.append(dict(xg=xg))
    return common, per_batch, flags


def build_program(flags, num_devices=8, gelu_fn=None):
    """Emit the bass program. Returns the compiled Bacc."""
    L = flags["n_layers"]
    D = flags["d_model"]
    DH = D // 2            # channels per half
    NK = max(1, D // 128)  # K tiles in W_out GEMM
    assert D % 2 == 0

    if gelu_fn is None:
        gelu_fn = AF.Gelu_apprx_tanh
    nc = bacc.Bacc("TRN2", target_bir_lowering=False, debug=False,
                   num_devices=num_devices)

    def din(name, shape, dt):
        return nc.dram_tensor(name, shape, dt, kind="ExternalInput").ap()

    xg = din("xg", [2, H * W], BF)
    wenc = din("wenc", [2, D], BF)
    benc_rep = din("benc_rep", [128, D], F32)
    tht = din("tht", [L, D, 128, 128], BF)
    twt = din("twt", [L, D, 128, 128], BF)
    drep = din("drep", [L, 128, D], F32)
    wout = din("wout", [L, NK, min(D, 128), 2 * D], BF)
    wdec_rep = din("wdec_rep", [128, D], BF)
    swd_rep = din("swd_rep", [128, 1], F32)
    ident = din("ident", [128, 128], BF)
    if flags["use_ln_affine"]:
        lnw_rep = din("lnw_rep", [L, 128, D], BF)
        lnb_rep = din("lnb_rep", [L, 128, D], BF)
    if flags["use_b_out"]:
        bout_rep = din("bout_rep", [L, 128, 2 * D], F32)
    out = nc.dram_tensor("out", [H, W], F32, kind="ExternalOutput").ap()

    from contextlib import ExitStack
    with tile.TileContext(nc) as tc, ExitStack() as ctx:
        state = ctx.enter_context(tc.tile_pool(name="state", bufs=1))
        consts = ctx.enter_context(tc.tile_pool(name="consts", bufs=1))
        wring = ctx.enter_context(tc.tile_pool(name="wring", bufs=20))
        lring = ctx.enter_context(tc.tile_pool(name="lring", bufs=2))
        atring = ctx.enter_context(tc.tile_pool(name="atring", bufs=3))
        cring = ctx.enter_context(tc.tile_pool(name="cring", bufs=3))
        gring = ctx.enter_context(tc.tile_pool(name="gring", bufs=2))
        sring = ctx.enter_context(tc.tile_pool(name="sring", bufs=3))
        stats = ctx.enter_context(tc.tile_pool(name="stats", bufs=1))
        ps = ctx.enter_context(tc.tile_pool(name="ps", bufs=8, space="PSUM"))

        h_sb = state.tile([128, W, D], BF, tag="h")
        Ysb = state.tile([128, DH, W], BF, tag="y")
        Xts = [state.tile([128, H * W], BF, tag=f"xt{k}", name=f"xt{k}")
               for k in range(NK)]

        wenc_t = consts.tile([2, D], BF)
        nc.sync.dma_start(wenc_t[:], wenc[:])
        benc_t = consts.tile([128, D], F32)
        nc.sync.dma_start(benc_t[:], benc_rep[:])
        ident_t = consts.tile([128, 128], BF)
        nc.sync.dma_start(ident_t[:], ident[:])
        wdec_t = consts.tile([128, D], BF)
        nc.sync.dma_start(wdec_t[:], wdec_rep[:])
        swd_t = consts.tile([128, 1], F32)
        nc.sync.dma_start(swd_t[:], swd_rep[:])
        eps_t = consts.tile([128, 1], F32)
        nc.vector.memset(eps_t[:], 1e-5)

        # ---------------- encoder ----------------
        for w0 in range(W):
            xg_t = wring.tile([2, 128], BF, tag="xg")
            nc.sync.dma_start(xg_t[:], xg[:, w0 * 128:(w0 + 1) * 128])
            psE = ps.tile([128, 2 * D], F32, tag="ps")
            nc.tensor.matmul(psE[:, 0:D], xg_t[:], wenc_t[:],
                             start=True, stop=True)
            nc.vector.scalar_tensor_tensor(
                out=h_sb[:, w0, :], in0=psE[:, 0:D], scalar=1.0,
                in1=benc_t[:], op0=OP.mult, op1=OP.add)

        # ---------------- layers ----------------
        for l in range(L):
            wout_ts = []
            for k in range(NK):
                wt = lring.tile([min(D, 128), 2 * D], BF, tag="woutw")
                nc.sync.dma_start(wt[:], wout[l, k])
                wout_ts.append(wt)
            drep_t = lring.tile([128, D], F32, tag="drep")
            nc.sync.dma_start(drep_t[:], drep[l])
            if flags["use_ln_affine"]:
                lnw_t = lring.tile([128, D], BF, tag="lnw")
                nc.sync.dma_start(lnw_t[:], lnw_rep[l])
                lnb_t = lring.tile([128, D], BF, tag="lnb")
                nc.sync.dma_start(lnb_t[:], lnb_rep[l])
            if flags["use_b_out"]:
                bout_t = lring.tile([128, 2 * D], F32, tag="bout")
                nc.sync.dma_start(bout_t[:], bout_rep[l])

            for hf in range(2):
                # ---- stage A: convolutions, 2 pairs (4 channels) at a time
                for dm in range(0, DH, 4):
                    d = hf * DH + dm
                    thts, twts = [], []
                    for j in range(4):
                        tt_ = wring.tile([128, 128], BF, tag="tht")
                        nc.sync.dma_start(tt_[:], tht[l, d + j])
                        thts.append(tt_)
                        tw_ = wring.tile([128, 128], BF, tag="twt")
                        nc.sync.dma_start(tw_[:], twt[l, d + j])
                        twts.append(tw_)

                    slot1 = ps.tile([128, 4, 128], F32, tag="ps")
                    for j in range(4):
                        nc.tensor.matmul(slot1[:, j, :], h_sb[:, :, d + j],
                                         thts[j][:], start=True, stop=True)
                    At4 = atring.tile([128, 4, 128], BF, tag="at")
                    nc.scalar.copy(At4[:], slot1[:])

                    slot2 = ps.tile([128, 4, 128], F32, tag="ps")
                    for j in range(4):
                        nc.tensor.matmul(slot2[:, j, :], At4[:, j, :],
                                         twts[j][:], start=True, stop=True)
                    # D-skip fused with copy-out to Ysb (channel-major)
                    for j in range(4):
                        nc.vector.scalar_tensor_tensor(
                            out=Ysb[:, dm + j, :], in0=h_sb[:, :, d + j],
                            scalar=drep_t[:, d + j:d + j + 1],
                            in1=slot2[:, j, :], op0=OP.mult, op1=OP.add)

                # ---- stage B: PE transpose + gelu into Xt ----
                for w0 in range(0, W, 4):
                    pst = ps.tile([128, 4, 128], BF, tag="ps")
                    for i in range(4):
                        nc.tensor.transpose(pst[:, i, :], Ysb[:, :, w0 + i],
                                            ident_t[:])
                    nc.scalar.activation(
                        Xts[hf][:, w0 * 128:(w0 + 4) * 128],
                        pst[:].rearrange("p a b -> p (a b)"), gelu_fn)

            # ---- stage C: W_out GEMM + GLU + residual + stats ----
            ssum = stats.tile([128, W], F32, tag="ssum")
            sqs = stats.tile([128, W], F32, tag="sqs")
            mu = stats.tile([128, W], F32, tag="mu")
            var = stats.tile([128, W], F32, tag="var")
            std = stats.tile([128, W], F32, tag="std")
            rr = stats.tile([128, W], F32, tag="rr")
            nmr = stats.tile([128, W], F32, tag="nmr")
            RB = 8  # residual batch (w0 per batched residual add)
            glu_big = None
            for wv in range(W):
                if wv % RB == 0:
                    glu_big = gring.tile([128, RB, D], BF, tag="glu")
                psW = ps.tile([128, 2 * D], F32, tag="ps")
                for k in range(NK):
                    nc.tensor.matmul(
                        psW[:],
                        Xts[k][:, wv * 128:(wv + 1) * 128],
                        wout_ts[k][:], start=(k == 0),
                        stop=(k == NK - 1))
                if flags["use_b_out"]:
                    nc.vector.tensor_tensor(psW[:], psW[:], bout_t[:],
                                            op=OP.add)
                # t1 = tanh(g/2); glu = (t1+1)*a'  (a' = 0.5*a baked in W_out)
                t1 = cring.tile([128, D], BF, tag="t1")
                nc.scalar.activation(t1[:], psW[:, D:2 * D], AF.Tanh,
                                     scale=0.5)
                g_sl = glu_big[:, wv % RB, :]
                if l > 0:
                    # sum_d h == 0 (LN output) => ssum accumulates on glu
                    nc.vector.scalar_tensor_tensor(
                        out=g_sl, in0=t1[:], scalar=1.0, in1=psW[:, 0:D],
                        op0=OP.add, op1=OP.mult,
                        accum_out=ssum[:, wv:wv + 1])
                else:
                    nc.vector.scalar_tensor_tensor(
                        out=g_sl, in0=t1[:], scalar=1.0, in1=psW[:, 0:D],
                        op0=OP.add, op1=OP.mult)
                if wv % RB == RB - 1:
                    b0 = wv - RB + 1
                    if l > 0:
                        # batched residual: h += glu
                        nc.vector.tensor_tensor(
                            h_sb[:, b0:wv + 1, :].rearrange("p a b -> p (a b)"),
                            glu_big[:].rearrange("p a b -> p (a b)"),
                            h_sb[:, b0:wv + 1, :].rearrange("p a b -> p (a b)"),
                            op=OP.add)
                    else:
                        for wx in range(b0, wv + 1):
                            nc.vector.scalar_tensor_tensor(
                                out=h_sb[:, wx, :], in0=glu_big[:, wx % RB, :],
                                scalar=1.0, in1=h_sb[:, wx, :],
                                op0=OP.mult, op1=OP.add,
                                accum_out=ssum[:, wx:wx + 1])
                    # sumsq per w0 (alternate DVE/ACT)
                    for wx in range(b0, wv + 1):
                        scr = cring.tile([128, D], BF, tag="scr")
                        if wx % 2 == 0:
                            nc.vector.scalar_tensor_tensor(
                                out=scr[:], in0=h_sb[:, wx, :], scalar=1.0,
                                in1=h_sb[:, wx, :], op0=OP.mult, op1=OP.mult,
                                accum_out=sqs[:, wx:wx + 1])
                        else:
                            nc.scalar.activation(
                                scr[:], h_sb[:, wx, :], AF.Square,
                                accum_out=sqs[:, wx:wx + 1])

            # ---- batched stats ----
            nc.vector.tensor_scalar(out=mu[:], in0=ssum[:],
                                    scalar1=1.0 / D, scalar2=None,
                                    op0=OP.mult)
            nc.vector.tensor_tensor(var[:], mu[:], mu[:], op=OP.mult)
            nc.vector.scalar_tensor_tensor(
                out=var[:], in0=sqs[:], scalar=1.0 / D,
                in1=var[:], op0=OP.mult, op1=OP.subtract)
            nc.scalar.activation(std[:], var[:], AF.Sqrt, bias=eps_t[:, 0:1])
            nc.vector.reciprocal(rr[:], std[:])
            nc.vector.scalar_tensor_tensor(
                out=nmr[:], in0=mu[:], scalar=-1.0,
                in1=rr[:], op0=OP.mult, op1=OP.mult)

            # ---- normalize pass (2/3 DVE, 1/3 ACT) ----
            # last layer: normalize is folded into the decoder instead
            if l == L - 1 and not flags["use_ln_affine"]:
                continue
            for w0 in range(W):
                if w0 % 8 == 4:
                    # keep-warm: idle PE re-throttles to 1.2 GHz after ~3.4us;
                    # a cheap transpose per 8 w0 keeps the HAM busy window hot
                    warm = ps.tile([128, 128], BF, tag="ps")
                    nc.tensor.transpose(warm[:], ident_t[:], ident_t[:])
                if w0 % 3 != 2:
                    nc.vector.tensor_scalar(
                        out=h_sb[:, w0, :], in0=h_sb[:, w0, :],
                        scalar1=rr[:, w0:w0 + 1], scalar2=nmr[:, w0:w0 + 1],
                        op0=OP.mult, op1=OP.add)
                else:
                    nc.scalar.activation(
                        h_sb[:, w0, :], h_sb[:, w0, :], AF.Identity,
                        bias=nmr[:, w0:w0 + 1], scale=rr[:, w0:w0 + 1])
                if flags["use_ln_affine"]:
                    nc.vector.tensor_tensor(
                        h_sb[:, w0, :], h_sb[:, w0, :], lnw_t[:],
                        op=OP.mult)
                    nc.vector.tensor_tensor(
                        h_sb[:, w0, :], h_sb[:, w0, :], lnb_t[:],
                        op=OP.add)

        # ---------------- decoder ----------------
        dec_sb = consts.tile([128, W], F32)
        for w0 in range(W):
            scr = cring.tile([128, D], BF, tag="scr")
            nc.vector.scalar_tensor_tensor(
                out=scr[:], in0=h_sb[:, w0, :], scalar=1.0, in1=wdec_t[:],
                op0=OP.mult, op1=OP.mult, accum_out=dec_sb[:, w0:w0 + 1])
        if not flags["use_ln_affine"]:
            # h held pre-normalize p: out = dec*rr + nmr*sum(wdec)
            nc.vector.tensor_tensor(dec_sb[:], dec_sb[:], rr[:], op=OP.mult)
            nc.vector.scalar_tensor_tensor(
                out=dec_sb[:], in0=nmr[:], scalar=swd_t[:, 0:1], in1=dec_sb[:],
                op0=OP.mult, op1=OP.add)
        if flags["b_dec"] != 0.0:
            nc.vector.tensor_scalar(out=dec_sb[:], in0=dec_sb[:],
                                    scalar1=float(flags["b_dec"]), scalar2=None,
                                    op0=OP.add)
        nc.sync.dma_start(out[:], dec_sb[:])

    nc.compile()
    return nc


# ---------------------------------------------------------------------------
# Self-contained entry point: full inputs in, full output out.
# Shards batch-parallel across 8 NeuronCores (cores 4..7 duplicate work).
# ---------------------------------------------------------------------------

_PROGRAM_CACHE = {}


def _get_program(flags):
    key = (flags["n_layers"], flags["d_model"], flags["use_ln_affine"],
           flags["use_b_out"], flags["b_dec"])
    if key not in _PROGRAM_CACHE:
        _PROGRAM_CACHE[key] = build_program(flags, num_devices=8)
    return _PROGRAM_CACHE[key]


def kernel(**inputs):
    import os
    from concourse.bass_utils import run_bass_kernel_spmd

    common, per_batch, flags = host_prep(inputs)
    nc = _get_program(flags)

    B = len(per_batch)
    in_maps = []
    for c in range(8):
        m = dict(common)
        m.update(per_batch[c % B])
        in_maps.append(m)

    trace = bool(os.environ.get("S4ND_TRACE"))
    res = run_bass_kernel_spmd(nc, in_maps, core_ids=list(range(8)), trace=trace)
    if trace and res.exec_time_ns is not None:
        print(f"HW exec time: {res.exec_time_ns} ns")
        kernel.last_exec_time_ns = res.exec_time_ns
        kernel.last_results = res

    out = np.stack([res.results[b]["out"] for b in range(B)], axis=0)[..., None]
    return out.astype(np.float32)


# revision 17
# speedup vs baseline: 2.1266x; 1.0118x over previous
"""S4ND Darcy-flow Bass kernel v3: builder + host-side preparation.

Design (per core = one batch element, batch-parallel over 4 cores, cores
4..7 duplicate work and are ignored at gather time):

  state h_sb: SBUF bf16 [128p=h, (w, d)], d innermost.
  Per layer, per half hf (128 channels each):
    stage A (conv), per 2-pair group (4 channels), one 8-bank psum pool:
      slot1 <- MM1 x4: A^T[w, h'] = U_d^T @ ThT_d
      copy1 (ACT, FD=512): slot1 -> At4 bf16
      slot2 <- MM2 x4 + MMd x4 (accumulate):
               y[h', w'] = At_d @ TwT_d + (D_d I) @ U_d   (D-skip on PE,
               dI tiles shipped from host)
      Ycopy (DVE, FD=512): slot2 -> Ysb[:, w, 4ch] (w-major [128, W, DH])
    stage B, per 4-w0 quad:
      DMA transpose x4: Ysb[:, w0+i, :] -> stg [dm, 4, h]  (DMA xbar engine)
      gelu (ACT, FD=512): stg -> Xt[hf]
  stage C, per w0 (8-deep psum pipeline):
    W_out GEMM x2 (k chunks) into psW [128, 512]
    tanh (ACT): t1 = tanh(0.5*g)         [sigmoid via tanh: same act table]
    glu (DVE stt + accum): glu = (t1+1)*a', accum ssum  (for layers >= 1,
        sum_d h == 0 exactly since h is LayerNorm output, so sum_d p =
        sum_d glu; layer 0 accumulates on the residual op instead)
    sumsq (DVE/ACT alternating, + accum sqs)
  residual (DVE tt, batched FD=2048 per 8 w0): h_sb += glu_big
  stats (batched, per layer): mu/var (DVE), std=sqrt(var+eps) (ACT, one
    table switch), rr=1/std (DVE recip), nmr=-mu*rr (DVE)
  normalize, per w0 (DVE/ACT alternating, per-partition AP scalars):
    h_sb = p*rr + nmr
  Decoder: DVE stt dot-products per w slice -> out (h, w) f32.

Host precomputes (numpy, float64): S4D kernels kh/kw, transposed Toeplitz
matrices ThT/TwT, D*I diagonal tiles, replicated small tensors, xg packing.
W_out a-half is pre-scaled by 0.5 for the tanh-based GLU.
"""

import numpy as np
import ml_dtypes

import concourse.bacc as bacc
import concourse.mybir as mybir
import concourse.tile as tile

bf16 = ml_dtypes.bfloat16
AF = mybir.ActivationFunctionType
OP = mybir.AluOpType
F32 = mybir.dt.float32
BF = mybir.dt.bfloat16

H = 128
W = 128


def host_prep(inputs, n_layers=None, d_model=None):
    """Compute device-side constant tensors from the full model inputs."""
    log_dt = np.asarray(inputs["log_dt"], np.float64)     # (L,2,d)
    logA_re = np.asarray(inputs["logA_re"], np.float64)   # (L,2,d,N)
    A_im = np.asarray(inputs["A_im"], np.float64)
    C_re = np.asarray(inputs["C_re"], np.float64)
    C_im = np.asarray(inputs["C_im"], np.float64)
    Dskip = np.asarray(inputs["Dskip"], np.float64)       # (L,d)
    W_out = np.asarray(inputs["W_out"], np.float64)       # (L,d,2d)
    b_out = np.asarray(inputs["b_out"], np.float64)       # (L,2d)
    ln_w = np.asarray(inputs["ln_w"], np.float64)         # (L,d)
    ln_b = np.asarray(inputs["ln_b"], np.float64)
    W_enc = np.asarray(inputs["W_enc"], np.float64)       # (2,d)
    b_enc = np.asarray(inputs["b_enc"], np.float64)       # (d,)
    W_dec = np.asarray(inputs["W_dec"], np.float64)       # (d,1)
    b_dec = np.asarray(inputs["b_dec"], np.float64)       # (1,)
    x = np.asarray(inputs["x"], np.float32)               # (B,H,W,1)
    grid = np.asarray(inputs["grid"], np.float32)

    L = log_dt.shape[0] if n_layers is None else n_layers
    D = log_dt.shape[2] if d_model is None else d_model
    log_dt = log_dt[:L, :, :D]
    logA_re = logA_re[:L, :, :D]
    A_im = A_im[:L, :, :D]
    C_re = C_re[:L, :, :D]
    C_im = C_im[:L, :, :D]
    Dskip = Dskip[:L, :D]
    d_full = W_out.shape[1]
    Wa = W_out[:L, :D, :D] * 0.5          # pre-scale a-half for tanh GLU
    Wg = W_out[:L, :D, d_full:d_full + D]
    W_out2 = np.concatenate([Wa, Wg], axis=2)             # (L, D, 2D)
    b_out2 = np.concatenate([b_out[:L, :D] * 0.5,
                             b_out[:L, d_full:d_full + D]], axis=1)
    ln_w = ln_w[:L, :D]
    ln_b = ln_b[:L, :D]
    W_enc = W_enc[:, :D]
    b_enc = b_enc[:D]
    W_dec = W_dec[:D]

    # ---- S4D kernels ----
    dt = np.exp(log_dt)[..., None]                        # (L,2,D,1)
    A = -np.exp(logA_re) + 1j * A_im                      # (L,2,D,N)
    C = C_re + 1j * C_im
    dtA = dt * A
    CB = C * (np.exp(dtA) - 1.0) / A
    t = np.arange(H, dtype=np.float64)
    pows = np.exp(dtA[..., None] * t)                     # (L,2,D,N,H)
    K = 2.0 * np.real(np.einsum("lxdn,lxdnt->lxdt", CB, pows))  # (L,2,D,H)
    kh = K[:, 0]                                          # (L,D,H)
    kw = K[:, 1]                                          # (L,D,W)

    # transposed lower-triangular Toeplitz: ThT[l,d,i,p] = kh[l,d,p-i], p>=i
    idx = np.arange(H)[None, :] - np.arange(H)[:, None]   # (i,p) = p-i
    mask = idx >= 0
    idxc = np.clip(idx, 0, H - 1)
    ThT = np.where(mask, kh[:, :, idxc], 0.0)             # (L,D,128,128)
    TwT = np.where(mask, kw[:, :, idxc], 0.0)

    flags = dict(
        use_ln_affine=not (np.all(ln_w == 1.0) and np.all(ln_b == 0.0)),
        use_b_out=not np.all(b_out2 == 0.0),
        n_layers=L,
        d_model=D,
        b_dec=float(b_dec[0]),
    )

    common = dict(
        wenc=W_enc.astype(np.float32).astype(bf16),                       # (2,D)
        benc_rep=np.tile(b_enc.astype(np.float32)[None, :], (128, 1)),    # (128,D) f32
        tht=ThT.astype(np.float32).astype(bf16),                          # (L,D,128,128)
        twt=TwT.astype(np.float32).astype(bf16),
        drep=np.tile(Dskip.astype(np.float32)[:, None, :], (1, 128, 1)),  # (L,128,D) f32
        wdec_rep=np.tile(W_dec.astype(np.float32).reshape(1, D), (128, 1)).astype(bf16),
        swd_rep=np.full((128, 1), float(np.sum(W_dec)), np.float32),
        ident=np.eye(128, dtype=np.float32).astype(bf16),
    )
    nk = max(1, D // 128)
    common["wout"] = np.ascontiguousarray(
        W_out2.reshape(L, nk, min(D, 128), 2 * D).astype(np.float32).astype(bf16)
    )
    if flags["use_ln_affine"]:
        common["lnw_rep"] = np.tile(ln_w.astype(np.float32)[:, None, :], (1, 128, 1)).astype(bf16)
        common["lnb_rep"] = np.tile(ln_b.astype(np.float32)[:, None, :], (1, 128, 1)).astype(bf16)
    if flags["use_b_out"]:
        common["bout_rep"] = np.tile(b_out2.astype(np.float32)[:, None, :], (1, 128, 1))

    per_batch = []
    for b in range(x.shape[0]):
        # xg[0, w*128+h] = x[b,h,w];  xg[1,...] = grid
        xb = x[b, :, :, 0].T.reshape(-1)     # (w,h) order
        gb = grid[b, :, :, 0].T.reshape(-1)
        xg = np.stack([xb, gb], axis=0).astype(np.float32).astype(bf16)
        per_batch.append(dict(xg=xg))
    return common, per_batch, flags


def build_program(flags, num_devices=8, gelu_fn=None):
    """Emit the bass program. Returns the compiled Bacc."""
    L = flags["n_layers"]
    D = flags["d_model"]
    DH = D // 2            # channels per half
    NK = max(1, D // 128)  # K tiles in W_out GEMM
    assert D % 2 == 0

    if gelu_fn is None:
        gelu_fn = AF.Gelu_apprx_tanh
    nc = bacc.Bacc("TRN2", target_bir_lowering=False, debug=False,
                   num_devices=num_devices)

    def din(name, shape, dt):
        return nc.dram_tensor(name, shape, dt, kind="ExternalInput").ap()

    xg = din("xg", [2, H * W], BF)
    wenc = din("wenc", [2, D], BF)
    benc_rep = din("benc_rep", [128, D], F32)
    tht = din("tht", [L, D, 128, 128], BF)
    twt = din("twt", [L, D, 128, 128], BF)
    drep = din("drep", [L, 128, D], F32)
    wout = din("wout", [L, NK, min(D, 128), 2 * D], BF)
    wdec_rep = din("wdec_rep", [128, D], BF)
    swd_rep = din("swd_rep", [128, 1], F32)
    ident = din("ident", [128, 128], BF)
    if flags["use_ln_affine"]:
        lnw_rep = din("lnw_rep", [L, 128, D], BF)
        lnb_rep = din("lnb_rep", [L, 128, D], BF)
    if flags["use_b_out"]:
        bout_rep = din("bout_rep", [L, 128, 2 * D], F32)
    out = nc.dram_tensor("out", [H, W], F32, kind="ExternalOutput").ap()

    from contextlib import ExitStack
    with tile.TileContext(nc) as tc, ExitStack() as ctx:
        state = ctx.enter_context(tc.tile_pool(name="state", bufs=1))
        consts = ctx.enter_context(tc.tile_pool(name="consts", bufs=1))
        wring = ctx.enter_context(tc.tile_pool(name="wring", bufs=20))
        lring = ctx.enter_context(tc.tile_pool(name="lring", bufs=2))
        atring = ctx.enter_context(tc.tile_pool(name="atring", bufs=3))
        cring = ctx.enter_context(tc.tile_pool(name="cring", bufs=3))
        gring = ctx.enter_context(tc.tile_pool(name="gring", bufs=2))
        sring = ctx.enter_context(tc.tile_pool(name="sring", bufs=3))
        stats = ctx.enter_context(tc.tile_pool(name="stats", bufs=1))
        ps = ctx.enter_context(tc.tile_pool(name="ps", bufs=8, space="PSUM"))

        h_sb = state.tile([128, W, D], BF, tag="h")
        Ysb = state.tile([128, DH, W], BF, tag="y")
        Xts = [state.tile([128, H * W], BF, tag=f"xt{k}", name=f"xt{k}")
               for k in range(NK)]

        wenc_t = consts.tile([2, D], BF)
        nc.sync.dma_start(wenc_t[:], wenc[:])
        benc_t = consts.tile([128, D], F32)
        nc.sync.dma_start(benc_t[:], benc_rep[:])
        ident_t = consts.tile([128, 128], BF)
        nc.sync.dma_start(ident_t[:], ident[:])
        wdec_t = consts.tile([128, D], BF)
        nc.sync.dma_start(wdec_t[:], wdec_rep[:])
        swd_t = consts.tile([128, 1], F32)
        nc.sync.dma_start(swd_t[:], swd_rep[:])
        eps_t = consts.tile([128, 1], F32)
        nc.vector.memset(eps_t[:], 1e-5)

        # ---------------- encoder ----------------
        for w0 in range(W):
            xg_t = wring.tile([2, 128], BF, tag="xg")
            nc.sync.dma_start(xg_t[:], xg[:, w0 * 128:(w0 + 1) * 128])
            psE = ps.tile([128, 2 * D], F32, tag="ps")
            nc.tensor.matmul(psE[:, 0:D], xg_t[:], wenc_t[:],
                             start=True, stop=True)
            nc.vector.scalar_tensor_tensor(
                out=h_sb[:, w0, :], in0=psE[:, 0:D], scalar=1.0,
                in1=benc_t[:], op0=OP.mult, op1=OP.add)

        # ---------------- layers ----------------
        for l in range(L):
            wout_ts = []
            for k in range(NK):
                wt = lring.tile([min(D, 128), 2 * D], BF, tag="woutw")
                nc.sync.dma_start(wt[:], wout[l, k])
                wout_ts.append(wt)
            drep_t = lring.tile([128, D], F32, tag="drep")
            nc.sync.dma_start(drep_t[:], drep[l])
            if flags["use_ln_affine"]:
                lnw_t = lring.tile([128, D], BF, tag="lnw")
                nc.sync.dma_start(lnw_t[:], lnw_rep[l])
                lnb_t = lring.tile([128, D], BF, tag="lnb")
                nc.sync.dma_start(lnb_t[:], lnb_rep[l])
            if flags["use_b_out"]:
                bout_t = lring.tile([128, 2 * D], F32, tag="bout")
                nc.sync.dma_start(bout_t[:], bout_rep[l])

            NBLK = 32  # channels per deferred-normalize block
            for hf in range(2):
                # ---- stage A: convolutions, 2 pairs (4 channels) at a time,
                # interleaved with the PREVIOUS layer's normalize in channel
                # blocks so the PE never waits on a serial normalize tail
                for dm in range(0, DH, 4):
                    d = hf * DH + dm
                    if l > 0 and d % NBLK == 0:
                        blk = h_sb[:, :, d:d + NBLK]
                        rrb = rr[:, :].unsqueeze(2).broadcast_to(
                            [128, W, NBLK])
                        nmb = nmr[:, :].unsqueeze(2).broadcast_to(
                            [128, W, NBLK])
                        nc.vector.tensor_tensor(blk, blk, rrb, op=OP.mult)
                        nc.vector.tensor_tensor(blk, blk, nmb, op=OP.add)
                    thts, twts = [], []
                    for j in range(4):
                        tt_ = wring.tile([128, 128], BF, tag="tht")
                        nc.sync.dma_start(tt_[:], tht[l, d + j])
                        thts.append(tt_)
                        tw_ = wring.tile([128, 128], BF, tag="twt")
                        nc.sync.dma_start(tw_[:], twt[l, d + j])
                        twts.append(tw_)

                    slot1 = ps.tile([128, 4, 128], F32, tag="ps")
                    for j in range(4):
                        nc.tensor.matmul(slot1[:, j, :], h_sb[:, :, d + j],
                                         thts[j][:], start=True, stop=True)
                    At4 = atring.tile([128, 4, 128], BF, tag="at")
                    nc.scalar.copy(At4[:], slot1[:])

                    slot2 = ps.tile([128, 4, 128], F32, tag="ps")
                    for j in range(4):
                        nc.tensor.matmul(slot2[:, j, :], At4[:, j, :],
                                         twts[j][:], start=True, stop=True)
                    # D-skip fused with copy-out to Ysb (channel-major)
                    for j in range(4):
                        nc.vector.scalar_tensor_tensor(
                            out=Ysb[:, dm + j, :], in0=h_sb[:, :, d + j],
                            scalar=drep_t[:, d + j:d + j + 1],
                            in1=slot2[:, j, :], op0=OP.mult, op1=OP.add)

                # ---- stage B: PE transpose + gelu into Xt ----
                for w0 in range(0, W, 4):
                    pst = ps.tile([128, 4, 128], BF, tag="ps")
                    for i in range(4):
                        nc.tensor.transpose(pst[:, i, :], Ysb[:, :, w0 + i],
                                            ident_t[:])
                    nc.scalar.activation(
                        Xts[hf][:, w0 * 128:(w0 + 4) * 128],
                        pst[:].rearrange("p a b -> p (a b)"), gelu_fn)

            # ---- stage C: W_out GEMM + GLU + residual + stats ----
            ssum = stats.tile([128, W], F32, tag="ssum")
            sqs = stats.tile([128, W], F32, tag="sqs")
            mu = stats.tile([128, W], F32, tag="mu")
            var = stats.tile([128, W], F32, tag="var")
            std = stats.tile([128, W], F32, tag="std")
            rr = stats.tile([128, W], F32, tag="rr")
            nmr = stats.tile([128, W], F32, tag="nmr")
            RB = 8  # residual batch (w0 per batched residual add)
            glu_big = None
            for wv in range(W):
                if wv % RB == 0:
                    glu_big = gring.tile([128, RB, D], BF, tag="glu")
                psW = ps.tile([128, 2 * D], F32, tag="ps")
                for k in range(NK):
                    nc.tensor.matmul(
                        psW[:],
                        Xts[k][:, wv * 128:(wv + 1) * 128],
                        wout_ts[k][:], start=(k == 0),
                        stop=(k == NK - 1))
                if flags["use_b_out"]:
                    nc.vector.tensor_tensor(psW[:], psW[:], bout_t[:],
                                            op=OP.add)
                # t1 = tanh(g/2); glu = (t1+1)*a'  (a' = 0.5*a baked in W_out)
                t1 = cring.tile([128, D], BF, tag="t1")
                nc.scalar.activation(t1[:], psW[:, D:2 * D], AF.Tanh,
                                     scale=0.5)
                g_sl = glu_big[:, wv % RB, :]
                if l > 0:
                    # sum_d h == 0 (LN output) => ssum accumulates on glu
                    nc.vector.scalar_tensor_tensor(
                        out=g_sl, in0=t1[:], scalar=1.0, in1=psW[:, 0:D],
                        op0=OP.add, op1=OP.mult,
                        accum_out=ssum[:, wv:wv + 1])
                else:
                    nc.vector.scalar_tensor_tensor(
                        out=g_sl, in0=t1[:], scalar=1.0, in1=psW[:, 0:D],
                        op0=OP.add, op1=OP.mult)
                if wv % RB == RB - 1:
                    b0 = wv - RB + 1
                    if l > 0:
                        # batched residual: h += glu
                        nc.vector.tensor_tensor(
                            h_sb[:, b0:wv + 1, :].rearrange("p a b -> p (a b)"),
                            glu_big[:].rearrange("p a b -> p (a b)"),
                            h_sb[:, b0:wv + 1, :].rearrange("p a b -> p (a b)"),
                            op=OP.add)
                    else:
                        for wx in range(b0, wv + 1):
                            nc.vector.scalar_tensor_tensor(
                                out=h_sb[:, wx, :], in0=glu_big[:, wx % RB, :],
                                scalar=1.0, in1=h_sb[:, wx, :],
                                op0=OP.mult, op1=OP.add,
                                accum_out=ssum[:, wx:wx + 1])
                    # sumsq per w0 (alternate DVE/ACT)
                    for wx in range(b0, wv + 1):
                        scr = cring.tile([128, D], BF, tag="scr")
                        if wx % 2 == 0:
                            nc.vector.scalar_tensor_tensor(
                                out=scr[:], in0=h_sb[:, wx, :], scalar=1.0,
                                in1=h_sb[:, wx, :], op0=OP.mult, op1=OP.mult,
                                accum_out=sqs[:, wx:wx + 1])
                        else:
                            nc.scalar.activation(
                                scr[:], h_sb[:, wx, :], AF.Square,
                                accum_out=sqs[:, wx:wx + 1])

            # ---- batched stats ----
            nc.vector.tensor_scalar(out=mu[:], in0=ssum[:],
                                    scalar1=1.0 / D, scalar2=None,
                                    op0=OP.mult)
            nc.vector.tensor_tensor(var[:], mu[:], mu[:], op=OP.mult)
            nc.vector.scalar_tensor_tensor(
                out=var[:], in0=sqs[:], scalar=1.0 / D,
                in1=var[:], op0=OP.mult, op1=OP.subtract)
            nc.scalar.activation(std[:], var[:], AF.Sqrt, bias=eps_t[:, 0:1])
            nc.vector.reciprocal(rr[:], std[:])
            nc.vector.scalar_tensor_tensor(
                out=nmr[:], in0=mu[:], scalar=-1.0,
                in1=rr[:], op0=OP.mult, op1=OP.mult)

            if flags["use_ln_affine"]:
                # fallback: eager per-w0 normalize + affine (unused when
                # ln is identity, which host_prep detects)
                for w0 in range(W):
                    nc.vector.tensor_scalar(
                        out=h_sb[:, w0, :], in0=h_sb[:, w0, :],
                        scalar1=rr[:, w0:w0 + 1], scalar2=nmr[:, w0:w0 + 1],
                        op0=OP.mult, op1=OP.add)
                    nc.vector.tensor_tensor(
                        h_sb[:, w0, :], h_sb[:, w0, :], lnw_t[:], op=OP.mult)
                    nc.vector.tensor_tensor(
                        h_sb[:, w0, :], h_sb[:, w0, :], lnb_t[:], op=OP.add)

        # ---------------- decoder ----------------
        dec_sb = consts.tile([128, W], F32)
        for w0 in range(W):
            scr = cring.tile([128, D], BF, tag="scr")
            nc.vector.scalar_tensor_tensor(
                out=scr[:], in0=h_sb[:, w0, :], scalar=1.0, in1=wdec_t[:],
                op0=OP.mult, op1=OP.mult, accum_out=dec_sb[:, w0:w0 + 1])
        if not flags["use_ln_affine"]:
            # h held pre-normalize p: out = dec*rr + nmr*sum(wdec)
            nc.vector.tensor_tensor(dec_sb[:], dec_sb[:], rr[:], op=OP.mult)
            nc.vector.scalar_tensor_tensor(
                out=dec_sb[:], in0=nmr[:], scalar=swd_t[:, 0:1], in1=dec_sb[:],
                op0=OP.mult, op1=OP.add)
        if flags["b_dec"] != 0.0:
            nc.vector.tensor_scalar(out=dec_sb[:], in0=dec_sb[:],
                                    scalar1=float(flags["b_dec"]), scalar2=None,
                                    op0=OP.add)
        nc.sync.dma_start(out[:], dec_sb[:])

    nc.compile()
    return nc


# ---------------------------------------------------------------------------
# Self-contained entry point: full inputs in, full output out.
# Shards batch-parallel across 8 NeuronCores (cores 4..7 duplicate work).
# ---------------------------------------------------------------------------

_PROGRAM_CACHE = {}


def _get_program(flags):
    key = (flags["n_layers"], flags["d_model"], flags["use_ln_affine"],
           flags["use_b_out"], flags["b_dec"])
    if key not in _PROGRAM_CACHE:
        _PROGRAM_CACHE[key] = build_program(flags, num_devices=8)
    return _PROGRAM_CACHE[key]


def kernel(**inputs):
    import os
    from concourse.bass_utils import run_bass_kernel_spmd

    common, per_batch, flags = host_prep(inputs)
    nc = _get_program(flags)

    B = len(per_batch)
    in_maps = []
    for c in range(8):
        m = dict(common)
        m.update(per_batch[c % B])
        in_maps.append(m)

    trace = bool(os.environ.get("S4ND_TRACE"))
    res = run_bass_kernel_spmd(nc, in_maps, core_ids=list(range(8)), trace=trace)
    if trace and res.exec_time_ns is not None:
        print(f"HW exec time: {res.exec_time_ns} ns")
        kernel.last_exec_time_ns = res.exec_time_ns
        kernel.last_results = res

    out = np.stack([res.results[b]["out"] for b in range(B)], axis=0)[..., None]
    return out.astype(np.float32)


# revision 18
# speedup vs baseline: 2.1475x; 1.0098x over previous
"""S4ND Darcy-flow Bass kernel v3: builder + host-side preparation.

Design (per core = one batch element, batch-parallel over 4 cores, cores
4..7 duplicate work and are ignored at gather time):

  state h_sb: SBUF bf16 [128p=h, (w, d)], d innermost.
  Per layer, per half hf (128 channels each):
    stage A (conv), per 2-pair group (4 channels), one 8-bank psum pool:
      slot1 <- MM1 x4: A^T[w, h'] = U_d^T @ ThT_d
      copy1 (ACT, FD=512): slot1 -> At4 bf16
      slot2 <- MM2 x4 + MMd x4 (accumulate):
               y[h', w'] = At_d @ TwT_d + (D_d I) @ U_d   (D-skip on PE,
               dI tiles shipped from host)
      Ycopy (DVE, FD=512): slot2 -> Ysb[:, w, 4ch] (w-major [128, W, DH])
    stage B, per 4-w0 quad:
      DMA transpose x4: Ysb[:, w0+i, :] -> stg [dm, 4, h]  (DMA xbar engine)
      gelu (ACT, FD=512): stg -> Xt[hf]
  stage C, per w0 (8-deep psum pipeline):
    W_out GEMM x2 (k chunks) into psW [128, 512]
    tanh (ACT): t1 = tanh(0.5*g)         [sigmoid via tanh: same act table]
    glu (DVE stt + accum): glu = (t1+1)*a', accum ssum  (for layers >= 1,
        sum_d h == 0 exactly since h is LayerNorm output, so sum_d p =
        sum_d glu; layer 0 accumulates on the residual op instead)
    sumsq (DVE/ACT alternating, + accum sqs)
  residual (DVE tt, batched FD=2048 per 8 w0): h_sb += glu_big
  stats (batched, per layer): mu/var (DVE), std=sqrt(var+eps) (ACT, one
    table switch), rr=1/std (DVE recip), nmr=-mu*rr (DVE)
  normalize, per w0 (DVE/ACT alternating, per-partition AP scalars):
    h_sb = p*rr + nmr
  Decoder: DVE stt dot-products per w slice -> out (h, w) f32.

Host precomputes (numpy, float64): S4D kernels kh/kw, transposed Toeplitz
matrices ThT/TwT, D*I diagonal tiles, replicated small tensors, xg packing.
W_out a-half is pre-scaled by 0.5 for the tanh-based GLU.
"""

import numpy as np
import ml_dtypes

import concourse.bacc as bacc
import concourse.mybir as mybir
import concourse.tile as tile

bf16 = ml_dtypes.bfloat16
AF = mybir.ActivationFunctionType
OP = mybir.AluOpType
F32 = mybir.dt.float32
BF = mybir.dt.bfloat16

H = 128
W = 128


def host_prep(inputs, n_layers=None, d_model=None):
    """Compute device-side constant tensors from the full model inputs."""
    log_dt = np.asarray(inputs["log_dt"], np.float64)     # (L,2,d)
    logA_re = np.asarray(inputs["logA_re"], np.float64)   # (L,2,d,N)
    A_im = np.asarray(inputs["A_im"], np.float64)
    C_re = np.asarray(inputs["C_re"], np.float64)
    C_im = np.asarray(inputs["C_im"], np.float64)
    Dskip = np.asarray(inputs["Dskip"], np.float64)       # (L,d)
    W_out = np.asarray(inputs["W_out"], np.float64)       # (L,d,2d)
    b_out = np.asarray(inputs["b_out"], np.float64)       # (L,2d)
    ln_w = np.asarray(inputs["ln_w"], np.float64)         # (L,d)
    ln_b = np.asarray(inputs["ln_b"], np.float64)
    W_enc = np.asarray(inputs["W_enc"], np.float64)       # (2,d)
    b_enc = np.asarray(inputs["b_enc"], np.float64)       # (d,)
    W_dec = np.asarray(inputs["W_dec"], np.float64)       # (d,1)
    b_dec = np.asarray(inputs["b_dec"], np.float64)       # (1,)
    x = np.asarray(inputs["x"], np.float32)               # (B,H,W,1)
    grid = np.asarray(inputs["grid"], np.float32)

    L = log_dt.shape[0] if n_layers is None else n_layers
    D = log_dt.shape[2] if d_model is None else d_model
    log_dt = log_dt[:L, :, :D]
    logA_re = logA_re[:L, :, :D]
    A_im = A_im[:L, :, :D]
    C_re = C_re[:L, :, :D]
    C_im = C_im[:L, :, :D]
    Dskip = Dskip[:L, :D]
    d_full = W_out.shape[1]
    Wa = W_out[:L, :D, :D] * 0.5          # pre-scale a-half for tanh GLU
    Wg = W_out[:L, :D, d_full:d_full + D]
    W_out2 = np.concatenate([Wa, Wg], axis=2)             # (L, D, 2D)
    b_out2 = np.concatenate([b_out[:L, :D] * 0.5,
                             b_out[:L, d_full:d_full + D]], axis=1)
    ln_w = ln_w[:L, :D]
    ln_b = ln_b[:L, :D]
    W_enc = W_enc[:, :D]
    b_enc = b_enc[:D]
    W_dec = W_dec[:D]

    # ---- S4D kernels ----
    dt = np.exp(log_dt)[..., None]                        # (L,2,D,1)
    A = -np.exp(logA_re) + 1j * A_im                      # (L,2,D,N)
    C = C_re + 1j * C_im
    dtA = dt * A
    CB = C * (np.exp(dtA) - 1.0) / A
    t = np.arange(H, dtype=np.float64)
    pows = np.exp(dtA[..., None] * t)                     # (L,2,D,N,H)
    K = 2.0 * np.real(np.einsum("lxdn,lxdnt->lxdt", CB, pows))  # (L,2,D,H)
    kh = K[:, 0]                                          # (L,D,H)
    kw = K[:, 1]                                          # (L,D,W)

    # transposed lower-triangular Toeplitz: ThT[l,d,i,p] = kh[l,d,p-i], p>=i
    idx = np.arange(H)[None, :] - np.arange(H)[:, None]   # (i,p) = p-i
    mask = idx >= 0
    idxc = np.clip(idx, 0, H - 1)
    ThT = np.where(mask, kh[:, :, idxc], 0.0)             # (L,D,128,128)
    TwT = np.where(mask, kw[:, :, idxc], 0.0)

    flags = dict(
        use_ln_affine=not (np.all(ln_w == 1.0) and np.all(ln_b == 0.0)),
        use_b_out=not np.all(b_out2 == 0.0),
        n_layers=L,
        d_model=D,
        b_dec=float(b_dec[0]),
    )

    common = dict(
        wenc=W_enc.astype(np.float32).astype(bf16),                       # (2,D)
        benc_rep=np.tile(b_enc.astype(np.float32)[None, :], (128, 1)),    # (128,D) f32
        tht=ThT.astype(np.float32).astype(bf16),                          # (L,D,128,128)
        twt=TwT.astype(np.float32).astype(bf16),
        drep=np.tile(Dskip.astype(np.float32)[:, None, :], (1, 128, 1)),  # (L,128,D) f32
        wdec_rep=np.tile(W_dec.astype(np.float32).reshape(1, D), (128, 1)).astype(bf16),
        swd_rep=np.full((128, 1), float(np.sum(W_dec)), np.float32),
        ident=np.eye(128, dtype=np.float32).astype(bf16),
    )
    nk = max(1, D // 128)
    common["wout"] = np.ascontiguousarray(
        W_out2.reshape(L, nk, min(D, 128), 2 * D).astype(np.float32).astype(bf16)
    )
    if flags["use_ln_affine"]:
        common["lnw_rep"] = np.tile(ln_w.astype(np.float32)[:, None, :], (1, 128, 1)).astype(bf16)
        common["lnb_rep"] = np.tile(ln_b.astype(np.float32)[:, None, :], (1, 128, 1)).astype(bf16)
    if flags["use_b_out"]:
        common["bout_rep"] = np.tile(b_out2.astype(np.float32)[:, None, :], (1, 128, 1))

    per_batch = []
    for b in range(x.shape[0]):
        # xg[0, w*128+h] = x[b,h,w];  xg[1,...] = grid
        xb = x[b, :, :, 0].T.reshape(-1)     # (w,h) order
        gb = grid[b, :, :, 0].T.reshape(-1)
        xg = np.stack([xb, gb], axis=0).astype(np.float32).astype(bf16)
        per_batch.append(dict(xg=xg))
    return common, per_batch, flags


def build_program(flags, num_devices=8, gelu_fn=None):
    """Emit the bass program. Returns the compiled Bacc."""
    L = flags["n_layers"]
    D = flags["d_model"]
    DH = D // 2            # channels per half
    NK = max(1, D // 128)  # K tiles in W_out GEMM
    assert D % 2 == 0

    if gelu_fn is None:
        gelu_fn = AF.Gelu_apprx_tanh
    nc = bacc.Bacc("TRN2", target_bir_lowering=False, debug=False,
                   num_devices=num_devices)

    def din(name, shape, dt):
        return nc.dram_tensor(name, shape, dt, kind="ExternalInput").ap()

    xg = din("xg", [2, H * W], BF)
    wenc = din("wenc", [2, D], BF)
    benc_rep = din("benc_rep", [128, D], F32)
    tht = din("tht", [L, D, 128, 128], BF)
    twt = din("twt", [L, D, 128, 128], BF)
    drep = din("drep", [L, 128, D], F32)
    wout = din("wout", [L, NK, min(D, 128), 2 * D], BF)
    wdec_rep = din("wdec_rep", [128, D], BF)
    swd_rep = din("swd_rep", [128, 1], F32)
    ident = din("ident", [128, 128], BF)
    if flags["use_ln_affine"]:
        lnw_rep = din("lnw_rep", [L, 128, D], BF)
        lnb_rep = din("lnb_rep", [L, 128, D], BF)
    if flags["use_b_out"]:
        bout_rep = din("bout_rep", [L, 128, 2 * D], F32)
    out = nc.dram_tensor("out", [H, W], F32, kind="ExternalOutput").ap()

    from contextlib import ExitStack
    with tile.TileContext(nc) as tc, ExitStack() as ctx:
        state = ctx.enter_context(tc.tile_pool(name="state", bufs=1))
        consts = ctx.enter_context(tc.tile_pool(name="consts", bufs=1))
        wring = ctx.enter_context(tc.tile_pool(name="wring", bufs=20))
        lring = ctx.enter_context(tc.tile_pool(name="lring", bufs=2))
        atring = ctx.enter_context(tc.tile_pool(name="atring", bufs=3))
        cring = ctx.enter_context(tc.tile_pool(name="cring", bufs=3))
        gring = ctx.enter_context(tc.tile_pool(name="gring", bufs=2))
        sring = ctx.enter_context(tc.tile_pool(name="sring", bufs=3))
        stats = ctx.enter_context(tc.tile_pool(name="stats", bufs=1))
        ps = ctx.enter_context(tc.tile_pool(name="ps", bufs=8, space="PSUM"))

        h_sb = state.tile([128, W, D], BF, tag="h")
        Ysb = state.tile([128, DH, W], BF, tag="y")
        Xts = [state.tile([128, H * W], BF, tag=f"xt{k}", name=f"xt{k}")
               for k in range(NK)]

        wenc_t = consts.tile([2, D], BF)
        nc.sync.dma_start(wenc_t[:], wenc[:])
        benc_t = consts.tile([128, D], F32)
        nc.sync.dma_start(benc_t[:], benc_rep[:])
        ident_t = consts.tile([128, 128], BF)
        nc.sync.dma_start(ident_t[:], ident[:])
        wdec_t = consts.tile([128, D], BF)
        nc.sync.dma_start(wdec_t[:], wdec_rep[:])
        swd_t = consts.tile([128, 1], F32)
        nc.sync.dma_start(swd_t[:], swd_rep[:])
        eps_t = consts.tile([128, 1], F32)
        nc.vector.memset(eps_t[:], 1e-5)

        # ---------------- encoder ----------------
        for w0 in range(W):
            xg_t = wring.tile([2, 128], BF, tag="xg")
            nc.sync.dma_start(xg_t[:], xg[:, w0 * 128:(w0 + 1) * 128])
            psE = ps.tile([128, 2 * D], F32, tag="ps")
            nc.tensor.matmul(psE[:, 0:D], xg_t[:], wenc_t[:],
                             start=True, stop=True)
            nc.vector.scalar_tensor_tensor(
                out=h_sb[:, w0, :], in0=psE[:, 0:D], scalar=1.0,
                in1=benc_t[:], op0=OP.mult, op1=OP.add)

        # ---------------- layers ----------------
        for l in range(L):
            wout_ts = []
            for k in range(NK):
                wt = lring.tile([min(D, 128), 2 * D], BF, tag="woutw")
                nc.sync.dma_start(wt[:], wout[l, k])
                wout_ts.append(wt)
            drep_t = lring.tile([128, D], F32, tag="drep")
            nc.sync.dma_start(drep_t[:], drep[l])
            if flags["use_ln_affine"]:
                lnw_t = lring.tile([128, D], BF, tag="lnw")
                nc.sync.dma_start(lnw_t[:], lnw_rep[l])
                lnb_t = lring.tile([128, D], BF, tag="lnb")
                nc.sync.dma_start(lnb_t[:], lnb_rep[l])
            if flags["use_b_out"]:
                bout_t = lring.tile([128, 2 * D], F32, tag="bout")
                nc.sync.dma_start(bout_t[:], bout_rep[l])

            NBLK = 32  # channels per deferred-normalize block
            for hf in range(2):
                # ---- stage A: convolutions, 2 pairs (4 channels) at a time,
                # interleaved with the PREVIOUS layer's normalize in channel
                # blocks so the PE never waits on a serial normalize tail
                for dm in range(0, DH, 4):
                    d = hf * DH + dm
                    if l > 0 and d % NBLK == 0:
                        blk = h_sb[:, :, d:d + NBLK]
                        rrb = rr[:, :].unsqueeze(2).broadcast_to(
                            [128, W, NBLK])
                        nmb = nmr[:, :].unsqueeze(2).broadcast_to(
                            [128, W, NBLK])
                        nc.vector.tensor_tensor(blk, blk, rrb, op=OP.mult)
                        nc.vector.tensor_tensor(blk, blk, nmb, op=OP.add)
                    thts, twts = [], []
                    for j in range(4):
                        tt_ = wring.tile([128, 128], BF, tag="tht")
                        nc.sync.dma_start(tt_[:], tht[l, d + j])
                        thts.append(tt_)
                        tw_ = wring.tile([128, 128], BF, tag="twt")
                        nc.sync.dma_start(tw_[:], twt[l, d + j])
                        twts.append(tw_)

                    slot1 = ps.tile([128, 4, 128], F32, tag="ps")
                    for j in range(4):
                        nc.tensor.matmul(slot1[:, j, :], h_sb[:, :, d + j],
                                         thts[j][:], start=True, stop=True)
                    At4 = atring.tile([128, 4, 128], BF, tag="at")
                    nc.scalar.copy(At4[:], slot1[:])

                    slot2 = ps.tile([128, 4, 128], F32, tag="ps")
                    for j in range(4):
                        nc.tensor.matmul(slot2[:, j, :], At4[:, j, :],
                                         twts[j][:], start=True, stop=True)
                    # D-skip fused with copy-out to Ysb (channel-major)
                    for j in range(4):
                        nc.vector.scalar_tensor_tensor(
                            out=Ysb[:, dm + j, :], in0=h_sb[:, :, d + j],
                            scalar=drep_t[:, d + j:d + j + 1],
                            in1=slot2[:, j, :], op0=OP.mult, op1=OP.add)

                # ---- stage B: PE transpose + gelu into Xt ----
                # half 0: emitted here (overlaps stage A of half 1).
                # half 1: deferred -- interleaved with stage C per 4-w0 so
                # the transposes fill stage C's idle PE.
                if hf == 0:
                    for w0 in range(0, W, 4):
                        pst = ps.tile([128, 4, 128], BF, tag="ps")
                        for i in range(4):
                            nc.tensor.transpose(pst[:, i, :],
                                                Ysb[:, :, w0 + i], ident_t[:])
                        nc.scalar.activation(
                            Xts[hf][:, w0 * 128:(w0 + 4) * 128],
                            pst[:].rearrange("p a b -> p (a b)"), gelu_fn)

            # ---- stage C: W_out GEMM + GLU + residual + stats,
            # interleaved with half-1 transposes+gelu per 4-w0 group ----
            ssum = stats.tile([128, W], F32, tag="ssum")
            sqs = stats.tile([128, W], F32, tag="sqs")
            mu = stats.tile([128, W], F32, tag="mu")
            var = stats.tile([128, W], F32, tag="var")
            std = stats.tile([128, W], F32, tag="std")
            rr = stats.tile([128, W], F32, tag="rr")
            nmr = stats.tile([128, W], F32, tag="nmr")
            RB = 8  # residual batch (w0 per batched residual add)
            glu_big = None
            for wv in range(W):
                if wv % 4 == 0:
                    pst = ps.tile([128, 4, 128], BF, tag="ps")
                    for i in range(4):
                        nc.tensor.transpose(pst[:, i, :],
                                            Ysb[:, :, wv + i], ident_t[:])
                    nc.scalar.activation(
                        Xts[1][:, wv * 128:(wv + 4) * 128],
                        pst[:].rearrange("p a b -> p (a b)"), gelu_fn)
                if wv % RB == 0:
                    glu_big = gring.tile([128, RB, D], BF, tag="glu")
                psW = ps.tile([128, 2 * D], F32, tag="ps")
                for k in range(NK):
                    nc.tensor.matmul(
                        psW[:],
                        Xts[k][:, wv * 128:(wv + 1) * 128],
                        wout_ts[k][:], start=(k == 0),
                        stop=(k == NK - 1))
                if flags["use_b_out"]:
                    nc.vector.tensor_tensor(psW[:], psW[:], bout_t[:],
                                            op=OP.add)
                # t1 = tanh(g/2); glu = (t1+1)*a'  (a' = 0.5*a baked in W_out)
                t1 = cring.tile([128, D], BF, tag="t1")
                nc.scalar.activation(t1[:], psW[:, D:2 * D], AF.Tanh,
                                     scale=0.5)
                g_sl = glu_big[:, wv % RB, :]
                if l > 0:
                    # sum_d h == 0 (LN output) => ssum accumulates on glu
                    nc.vector.scalar_tensor_tensor(
                        out=g_sl, in0=t1[:], scalar=1.0, in1=psW[:, 0:D],
                        op0=OP.add, op1=OP.mult,
                        accum_out=ssum[:, wv:wv + 1])
                else:
                    nc.vector.scalar_tensor_tensor(
                        out=g_sl, in0=t1[:], scalar=1.0, in1=psW[:, 0:D],
                        op0=OP.add, op1=OP.mult)
                if wv % RB == RB - 1:
                    b0 = wv - RB + 1
                    if l > 0:
                        # batched residual: h += glu
                        nc.vector.tensor_tensor(
                            h_sb[:, b0:wv + 1, :].rearrange("p a b -> p (a b)"),
                            glu_big[:].rearrange("p a b -> p (a b)"),
                            h_sb[:, b0:wv + 1, :].rearrange("p a b -> p (a b)"),
                            op=OP.add)
                    else:
                        for wx in range(b0, wv + 1):
                            nc.vector.scalar_tensor_tensor(
                                out=h_sb[:, wx, :], in0=glu_big[:, wx % RB, :],
                                scalar=1.0, in1=h_sb[:, wx, :],
                                op0=OP.mult, op1=OP.add,
                                accum_out=ssum[:, wx:wx + 1])
                    # sumsq per w0 (alternate DVE/ACT)
                    for wx in range(b0, wv + 1):
                        scr = cring.tile([128, D], BF, tag="scr")
                        if wx % 2 == 0:
                            nc.vector.scalar_tensor_tensor(
                                out=scr[:], in0=h_sb[:, wx, :], scalar=1.0,
                                in1=h_sb[:, wx, :], op0=OP.mult, op1=OP.mult,
                                accum_out=sqs[:, wx:wx + 1])
                        else:
                            nc.scalar.activation(
                                scr[:], h_sb[:, wx, :], AF.Square,
                                accum_out=sqs[:, wx:wx + 1])

            # ---- batched stats ----
            nc.vector.tensor_scalar(out=mu[:], in0=ssum[:],
                                    scalar1=1.0 / D, scalar2=None,
                                    op0=OP.mult)
            nc.vector.tensor_tensor(var[:], mu[:], mu[:], op=OP.mult)
            nc.vector.scalar_tensor_tensor(
                out=var[:], in0=sqs[:], scalar=1.0 / D,
                in1=var[:], op0=OP.mult, op1=OP.subtract)
            nc.scalar.activation(std[:], var[:], AF.Sqrt, bias=eps_t[:, 0:1])
            nc.vector.reciprocal(rr[:], std[:])
            nc.vector.scalar_tensor_tensor(
                out=nmr[:], in0=mu[:], scalar=-1.0,
                in1=rr[:], op0=OP.mult, op1=OP.mult)

            if flags["use_ln_affine"]:
                # fallback: eager per-w0 normalize + affine (unused when
                # ln is identity, which host_prep detects)
                for w0 in range(W):
                    nc.vector.tensor_scalar(
                        out=h_sb[:, w0, :], in0=h_sb[:, w0, :],
                        scalar1=rr[:, w0:w0 + 1], scalar2=nmr[:, w0:w0 + 1],
                        op0=OP.mult, op1=OP.add)
                    nc.vector.tensor_tensor(
                        h_sb[:, w0, :], h_sb[:, w0, :], lnw_t[:], op=OP.mult)
                    nc.vector.tensor_tensor(
                        h_sb[:, w0, :], h_sb[:, w0, :], lnb_t[:], op=OP.add)

        # ---------------- decoder ----------------
        dec_sb = consts.tile([128, W], F32)
        for w0 in range(W):
            scr = cring.tile([128, D], BF, tag="scr")
            nc.vector.scalar_tensor_tensor(
                out=scr[:], in0=h_sb[:, w0, :], scalar=1.0, in1=wdec_t[:],
                op0=OP.mult, op1=OP.mult, accum_out=dec_sb[:, w0:w0 + 1])
        if not flags["use_ln_affine"]:
            # h held pre-normalize p: out = dec*rr + nmr*sum(wdec)
            nc.vector.tensor_tensor(dec_sb[:], dec_sb[:], rr[:], op=OP.mult)
            nc.vector.scalar_tensor_tensor(
                out=dec_sb[:], in0=nmr[:], scalar=swd_t[:, 0:1], in1=dec_sb[:],
                op0=OP.mult, op1=OP.add)
        if flags["b_dec"] != 0.0:
            nc.vector.tensor_scalar(out=dec_sb[:], in0=dec_sb[:],
                                    scalar1=float(flags["b_dec"]), scalar2=None,
                                    op0=OP.add)
        nc.sync.dma_start(out[:], dec_sb[:])

    nc.compile()
    return nc


# ---------------------------------------------------------------------------
# Self-contained entry point: full inputs in, full output out.
# Shards batch-parallel across 8 NeuronCores (cores 4..7 duplicate work).
# ---------------------------------------------------------------------------

_PROGRAM_CACHE = {}


def _get_program(flags):
    key = (flags["n_layers"], flags["d_model"], flags["use_ln_affine"],
           flags["use_b_out"], flags["b_dec"])
    if key not in _PROGRAM_CACHE:
        _PROGRAM_CACHE[key] = build_program(flags, num_devices=8)
    return _PROGRAM_CACHE[key]


def kernel(**inputs):
    import os
    from concourse.bass_utils import run_bass_kernel_spmd

    common, per_batch, flags = host_prep(inputs)
    nc = _get_program(flags)

    B = len(per_batch)
    in_maps = []
    for c in range(8):
        m = dict(common)
        m.update(per_batch[c % B])
        in_maps.append(m)

    trace = bool(os.environ.get("S4ND_TRACE"))
    res = run_bass_kernel_spmd(nc, in_maps, core_ids=list(range(8)), trace=trace)
    if trace and res.exec_time_ns is not None:
        print(f"HW exec time: {res.exec_time_ns} ns")
        kernel.last_exec_time_ns = res.exec_time_ns
        kernel.last_results = res

    out = np.stack([res.results[b]["out"] for b in range(B)], axis=0)[..., None]
    return out.astype(np.float32)


# revision 19
# speedup vs baseline: 2.1479x; 1.0002x over previous
"""S4ND Darcy-flow Bass kernel v3: builder + host-side preparation.

Design (per core = one batch element, batch-parallel over 4 cores, cores
4..7 duplicate work and are ignored at gather time):

  state h_sb: SBUF bf16 [128p=h, (w, d)], d innermost.
  Per layer, per half hf (128 channels each):
    stage A (conv), per 2-pair group (4 channels), one 8-bank psum pool:
      slot1 <- MM1 x4: A^T[w, h'] = U_d^T @ ThT_d
      copy1 (ACT, FD=512): slot1 -> At4 bf16
      slot2 <- MM2 x4 + MMd x4 (accumulate):
               y[h', w'] = At_d @ TwT_d + (D_d I) @ U_d   (D-skip on PE,
               dI tiles shipped from host)
      Ycopy (DVE, FD=512): slot2 -> Ysb[:, w, 4ch] (w-major [128, W, DH])
    stage B, per 4-w0 quad:
      DMA transpose x4: Ysb[:, w0+i, :] -> stg [dm, 4, h]  (DMA xbar engine)
      gelu (ACT, FD=512): stg -> Xt[hf]
  stage C, per w0 (8-deep psum pipeline):
    W_out GEMM x2 (k chunks) into psW [128, 512]
    tanh (ACT): t1 = tanh(0.5*g)         [sigmoid via tanh: same act table]
    glu (DVE stt + accum): glu = (t1+1)*a', accum ssum  (for layers >= 1,
        sum_d h == 0 exactly since h is LayerNorm output, so sum_d p =
        sum_d glu; layer 0 accumulates on the residual op instead)
    sumsq (DVE/ACT alternating, + accum sqs)
  residual (DVE tt, batched FD=2048 per 8 w0): h_sb += glu_big
  stats (batched, per layer): mu/var (DVE), std=sqrt(var+eps) (ACT, one
    table switch), rr=1/std (DVE recip), nmr=-mu*rr (DVE)
  normalize, per w0 (DVE/ACT alternating, per-partition AP scalars):
    h_sb = p*rr + nmr
  Decoder: DVE stt dot-products per w slice -> out (h, w) f32.

Host precomputes (numpy, float64): S4D kernels kh/kw, transposed Toeplitz
matrices ThT/TwT, D*I diagonal tiles, replicated small tensors, xg packing.
W_out a-half is pre-scaled by 0.5 for the tanh-based GLU.
"""

import numpy as np
import ml_dtypes

import concourse.bacc as bacc
import concourse.mybir as mybir
import concourse.tile as tile

bf16 = ml_dtypes.bfloat16
AF = mybir.ActivationFunctionType
OP = mybir.AluOpType
F32 = mybir.dt.float32
BF = mybir.dt.bfloat16

H = 128
W = 128


def host_prep(inputs, n_layers=None, d_model=None):
    """Compute device-side constant tensors from the full model inputs."""
    log_dt = np.asarray(inputs["log_dt"], np.float64)     # (L,2,d)
    logA_re = np.asarray(inputs["logA_re"], np.float64)   # (L,2,d,N)
    A_im = np.asarray(inputs["A_im"], np.float64)
    C_re = np.asarray(inputs["C_re"], np.float64)
    C_im = np.asarray(inputs["C_im"], np.float64)
    Dskip = np.asarray(inputs["Dskip"], np.float64)       # (L,d)
    W_out = np.asarray(inputs["W_out"], np.float64)       # (L,d,2d)
    b_out = np.asarray(inputs["b_out"], np.float64)       # (L,2d)
    ln_w = np.asarray(inputs["ln_w"], np.float64)         # (L,d)
    ln_b = np.asarray(inputs["ln_b"], np.float64)
    W_enc = np.asarray(inputs["W_enc"], np.float64)       # (2,d)
    b_enc = np.asarray(inputs["b_enc"], np.float64)       # (d,)
    W_dec = np.asarray(inputs["W_dec"], np.float64)       # (d,1)
    b_dec = np.asarray(inputs["b_dec"], np.float64)       # (1,)
    x = np.asarray(inputs["x"], np.float32)               # (B,H,W,1)
    grid = np.asarray(inputs["grid"], np.float32)

    L = log_dt.shape[0] if n_layers is None else n_layers
    D = log_dt.shape[2] if d_model is None else d_model
    log_dt = log_dt[:L, :, :D]
    logA_re = logA_re[:L, :, :D]
    A_im = A_im[:L, :, :D]
    C_re = C_re[:L, :, :D]
    C_im = C_im[:L, :, :D]
    Dskip = Dskip[:L, :D]
    d_full = W_out.shape[1]
    Wa = W_out[:L, :D, :D] * 0.5          # pre-scale a-half for tanh GLU
    Wg = W_out[:L, :D, d_full:d_full + D]
    W_out2 = np.concatenate([Wa, Wg], axis=2)             # (L, D, 2D)
    b_out2 = np.concatenate([b_out[:L, :D] * 0.5,
                             b_out[:L, d_full:d_full + D]], axis=1)
    ln_w = ln_w[:L, :D]
    ln_b = ln_b[:L, :D]
    W_enc = W_enc[:, :D]
    b_enc = b_enc[:D]
    W_dec = W_dec[:D]

    # ---- S4D kernels ----
    dt = np.exp(log_dt)[..., None]                        # (L,2,D,1)
    A = -np.exp(logA_re) + 1j * A_im                      # (L,2,D,N)
    C = C_re + 1j * C_im
    dtA = dt * A
    CB = C * (np.exp(dtA) - 1.0) / A
    t = np.arange(H, dtype=np.float64)
    pows = np.exp(dtA[..., None] * t)                     # (L,2,D,N,H)
    K = 2.0 * np.real(np.einsum("lxdn,lxdnt->lxdt", CB, pows))  # (L,2,D,H)
    kh = K[:, 0]                                          # (L,D,H)
    kw = K[:, 1]                                          # (L,D,W)

    # transposed lower-triangular Toeplitz: ThT[l,d,i,p] = kh[l,d,p-i], p>=i
    idx = np.arange(H)[None, :] - np.arange(H)[:, None]   # (i,p) = p-i
    mask = idx >= 0
    idxc = np.clip(idx, 0, H - 1)
    ThT = np.where(mask, kh[:, :, idxc], 0.0)             # (L,D,128,128)
    TwT = np.where(mask, kw[:, :, idxc], 0.0)

    flags = dict(
        use_ln_affine=not (np.all(ln_w == 1.0) and np.all(ln_b == 0.0)),
        use_b_out=not np.all(b_out2 == 0.0),
        n_layers=L,
        d_model=D,
        b_dec=float(b_dec[0]),
    )

    common = dict(
        wenc=W_enc.astype(np.float32).astype(bf16),                       # (2,D)
        benc_rep=np.tile(b_enc.astype(np.float32)[None, :], (128, 1)),    # (128,D) f32
        tht=ThT.astype(np.float32).astype(bf16),                          # (L,D,128,128)
        twt=TwT.astype(np.float32).astype(bf16),
        drep=np.tile(Dskip.astype(np.float32)[:, None, :], (1, 128, 1)),  # (L,128,D) f32
        wdec_rep=np.tile(W_dec.astype(np.float32).reshape(1, D), (128, 1)).astype(bf16),
        swd_rep=np.full((128, 1), float(np.sum(W_dec)), np.float32),
        ident=np.eye(128, dtype=np.float32).astype(bf16),
    )
    nk = max(1, D // 128)
    common["wout"] = np.ascontiguousarray(
        W_out2.reshape(L, nk, min(D, 128), 2 * D).astype(np.float32).astype(bf16)
    )
    if flags["use_ln_affine"]:
        common["lnw_rep"] = np.tile(ln_w.astype(np.float32)[:, None, :], (1, 128, 1)).astype(bf16)
        common["lnb_rep"] = np.tile(ln_b.astype(np.float32)[:, None, :], (1, 128, 1)).astype(bf16)
    if flags["use_b_out"]:
        common["bout_rep"] = np.tile(b_out2.astype(np.float32)[:, None, :], (1, 128, 1))

    per_batch = []
    for b in range(x.shape[0]):
        # xg[0, w*128+h] = x[b,h,w];  xg[1,...] = grid
        xb = x[b, :, :, 0].T.reshape(-1)     # (w,h) order
        gb = grid[b, :, :, 0].T.reshape(-1)
        xg = np.stack([xb, gb], axis=0).astype(np.float32).astype(bf16)
        per_batch.append(dict(xg=xg))
    return common, per_batch, flags


def build_program(flags, num_devices=8, gelu_fn=None):
    """Emit the bass program. Returns the compiled Bacc."""
    L = flags["n_layers"]
    D = flags["d_model"]
    DH = D // 2            # channels per half
    NK = max(1, D // 128)  # K tiles in W_out GEMM
    assert D % 2 == 0

    if gelu_fn is None:
        gelu_fn = AF.Gelu_apprx_tanh
    nc = bacc.Bacc("TRN2", target_bir_lowering=False, debug=False,
                   num_devices=num_devices)

    def din(name, shape, dt):
        return nc.dram_tensor(name, shape, dt, kind="ExternalInput").ap()

    xg = din("xg", [2, H * W], BF)
    wenc = din("wenc", [2, D], BF)
    benc_rep = din("benc_rep", [128, D], F32)
    tht = din("tht", [L, D, 128, 128], BF)
    twt = din("twt", [L, D, 128, 128], BF)
    drep = din("drep", [L, 128, D], F32)
    wout = din("wout", [L, NK, min(D, 128), 2 * D], BF)
    wdec_rep = din("wdec_rep", [128, D], BF)
    swd_rep = din("swd_rep", [128, 1], F32)
    ident = din("ident", [128, 128], BF)
    if flags["use_ln_affine"]:
        lnw_rep = din("lnw_rep", [L, 128, D], BF)
        lnb_rep = din("lnb_rep", [L, 128, D], BF)
    if flags["use_b_out"]:
        bout_rep = din("bout_rep", [L, 128, 2 * D], F32)
    out = nc.dram_tensor("out", [H, W], F32, kind="ExternalOutput").ap()

    from contextlib import ExitStack
    with tile.TileContext(nc) as tc, ExitStack() as ctx:
        state = ctx.enter_context(tc.tile_pool(name="state", bufs=1))
        consts = ctx.enter_context(tc.tile_pool(name="consts", bufs=1))
        wring = ctx.enter_context(tc.tile_pool(name="wring", bufs=20))
        lring = ctx.enter_context(tc.tile_pool(name="lring", bufs=2))
        atring = ctx.enter_context(tc.tile_pool(name="atring", bufs=3))
        cring = ctx.enter_context(tc.tile_pool(name="cring", bufs=3))
        gring = ctx.enter_context(tc.tile_pool(name="gring", bufs=2))
        sring = ctx.enter_context(tc.tile_pool(name="sring", bufs=3))
        stats = ctx.enter_context(tc.tile_pool(name="stats", bufs=1))
        ps = ctx.enter_context(tc.tile_pool(name="ps", bufs=8, space="PSUM"))

        h_sb = state.tile([128, W, D], BF, tag="h")
        Ysb = state.tile([128, DH, W], BF, tag="y")
        Xts = [state.tile([128, H * W], BF, tag=f"xt{k}", name=f"xt{k}")
               for k in range(NK)]

        wenc_t = consts.tile([2, D], BF)
        nc.sync.dma_start(wenc_t[:], wenc[:])
        benc_t = consts.tile([128, D], F32)
        nc.sync.dma_start(benc_t[:], benc_rep[:])
        ident_t = consts.tile([128, 128], BF)
        nc.sync.dma_start(ident_t[:], ident[:])
        wdec_t = consts.tile([128, D], BF)
        nc.sync.dma_start(wdec_t[:], wdec_rep[:])
        swd_t = consts.tile([128, 1], F32)
        nc.sync.dma_start(swd_t[:], swd_rep[:])
        eps_t = consts.tile([128, 1], F32)
        nc.vector.memset(eps_t[:], 1e-5)

        # ---------------- encoder ----------------
        for w0 in range(W):
            xg_t = wring.tile([2, 128], BF, tag="xg")
            nc.sync.dma_start(xg_t[:], xg[:, w0 * 128:(w0 + 1) * 128])
            psE = ps.tile([128, 2 * D], F32, tag="ps")
            nc.tensor.matmul(psE[:, 0:D], xg_t[:], wenc_t[:],
                             start=True, stop=True)
            nc.vector.scalar_tensor_tensor(
                out=h_sb[:, w0, :], in0=psE[:, 0:D], scalar=1.0,
                in1=benc_t[:], op0=OP.mult, op1=OP.add)

        # ---------------- layers ----------------
        for l in range(L):
            wout_ts = []
            for k in range(NK):
                wt = lring.tile([min(D, 128), 2 * D], BF, tag="woutw")
                nc.sync.dma_start(wt[:], wout[l, k])
                wout_ts.append(wt)
            drep_t = lring.tile([128, D], F32, tag="drep")
            nc.sync.dma_start(drep_t[:], drep[l])
            if flags["use_ln_affine"]:
                lnw_t = lring.tile([128, D], BF, tag="lnw")
                nc.sync.dma_start(lnw_t[:], lnw_rep[l])
                lnb_t = lring.tile([128, D], BF, tag="lnb")
                nc.sync.dma_start(lnb_t[:], lnb_rep[l])
            if flags["use_b_out"]:
                bout_t = lring.tile([128, 2 * D], F32, tag="bout")
                nc.sync.dma_start(bout_t[:], bout_rep[l])

            NBLK = 32  # channels per deferred-normalize block
            for hf in range(2):
                # ---- stage A: convolutions, 2 pairs (4 channels) at a time,
                # interleaved with the PREVIOUS layer's normalize in channel
                # blocks so the PE never waits on a serial normalize tail
                for dm in range(0, DH, 4):
                    d = hf * DH + dm
                    if l > 0 and d % NBLK == 0 and not flags["use_ln_affine"]:
                        blk = h_sb[:, :, d:d + NBLK]
                        rrb = rr[:, :].unsqueeze(2).broadcast_to(
                            [128, W, NBLK])
                        nmb = nmr[:, :].unsqueeze(2).broadcast_to(
                            [128, W, NBLK])
                        nc.vector.tensor_tensor(blk, blk, rrb, op=OP.mult)
                        nc.vector.tensor_tensor(blk, blk, nmb, op=OP.add)
                    thts, twts = [], []
                    for j in range(4):
                        tt_ = wring.tile([128, 128], BF, tag="tht")
                        nc.sync.dma_start(tt_[:], tht[l, d + j])
                        thts.append(tt_)
                        tw_ = wring.tile([128, 128], BF, tag="twt")
                        nc.sync.dma_start(tw_[:], twt[l, d + j])
                        twts.append(tw_)

                    slot1 = ps.tile([128, 4, 128], F32, tag="ps")
                    for j in range(4):
                        nc.tensor.matmul(slot1[:, j, :], h_sb[:, :, d + j],
                                         thts[j][:], start=True, stop=True)
                    At4 = atring.tile([128, 4, 128], BF, tag="at")
                    nc.scalar.copy(At4[:], slot1[:])

                    slot2 = ps.tile([128, 4, 128], F32, tag="ps")
                    for j in range(4):
                        nc.tensor.matmul(slot2[:, j, :], At4[:, j, :],
                                         twts[j][:], start=True, stop=True)
                    # D-skip fused with copy-out to Ysb (channel-major)
                    for j in range(4):
                        nc.vector.scalar_tensor_tensor(
                            out=Ysb[:, dm + j, :], in0=h_sb[:, :, d + j],
                            scalar=drep_t[:, d + j:d + j + 1],
                            in1=slot2[:, j, :], op0=OP.mult, op1=OP.add)

                # ---- stage B: PE transpose + gelu into Xt ----
                # half 0: emitted here (overlaps stage A of half 1).
                # half 1: deferred -- interleaved with stage C per 4-w0 so
                # the transposes fill stage C's idle PE.
                if hf == 0:
                    for w0 in range(0, W, 4):
                        pst = ps.tile([128, 4, 128], BF, tag="ps")
                        for i in range(4):
                            nc.tensor.transpose(pst[:, i, :],
                                                Ysb[:, :, w0 + i], ident_t[:])
                        nc.scalar.activation(
                            Xts[hf][:, w0 * 128:(w0 + 4) * 128],
                            pst[:].rearrange("p a b -> p (a b)"), gelu_fn)

            # ---- stage C: W_out GEMM + GLU + residual + stats,
            # interleaved with half-1 transposes+gelu per 4-w0 group ----
            ssum = stats.tile([128, W], F32, tag="ssum")
            sqs = stats.tile([128, W], F32, tag="sqs")
            mu = stats.tile([128, W], F32, tag="mu")
            var = stats.tile([128, W], F32, tag="var")
            std = stats.tile([128, W], F32, tag="std")
            rr = stats.tile([128, W], F32, tag="rr")
            nmr = stats.tile([128, W], F32, tag="nmr")
            RB = 8  # residual batch (w0 per batched residual add)
            glu_big = None
            for wv in range(W):
                if wv % 4 == 0:
                    pst = ps.tile([128, 4, 128], BF, tag="ps")
                    for i in range(4):
                        nc.tensor.transpose(pst[:, i, :],
                                            Ysb[:, :, wv + i], ident_t[:])
                    nc.scalar.activation(
                        Xts[1][:, wv * 128:(wv + 4) * 128],
                        pst[:].rearrange("p a b -> p (a b)"), gelu_fn)
                if wv % RB == 0:
                    glu_big = gring.tile([128, RB, D], BF, tag="glu")
                psW = ps.tile([128, 2 * D], F32, tag="ps")
                for k in range(NK):
                    nc.tensor.matmul(
                        psW[:],
                        Xts[k][:, wv * 128:(wv + 1) * 128],
                        wout_ts[k][:], start=(k == 0),
                        stop=(k == NK - 1))
                if flags["use_b_out"]:
                    nc.vector.tensor_tensor(psW[:], psW[:], bout_t[:],
                                            op=OP.add)
                # t1 = tanh(g/2); glu = (t1+1)*a'  (a' = 0.5*a baked in W_out)
                t1 = cring.tile([128, D], BF, tag="t1")
                nc.scalar.activation(t1[:], psW[:, D:2 * D], AF.Tanh,
                                     scale=0.5)
                g_sl = glu_big[:, wv % RB, :]
                if l > 0:
                    # sum_d h == 0 (LN output) => ssum accumulates on glu
                    nc.vector.scalar_tensor_tensor(
                        out=g_sl, in0=t1[:], scalar=1.0, in1=psW[:, 0:D],
                        op0=OP.add, op1=OP.mult,
                        accum_out=ssum[:, wv:wv + 1])
                else:
                    nc.vector.scalar_tensor_tensor(
                        out=g_sl, in0=t1[:], scalar=1.0, in1=psW[:, 0:D],
                        op0=OP.add, op1=OP.mult)
                if wv % RB == RB - 1:
                    b0 = wv - RB + 1
                    if l > 0:
                        # batched residual: h += glu
                        nc.vector.tensor_tensor(
                            h_sb[:, b0:wv + 1, :].rearrange("p a b -> p (a b)"),
                            glu_big[:].rearrange("p a b -> p (a b)"),
                            h_sb[:, b0:wv + 1, :].rearrange("p a b -> p (a b)"),
                            op=OP.add)
                    else:
                        for wx in range(b0, wv + 1):
                            nc.vector.scalar_tensor_tensor(
                                out=h_sb[:, wx, :], in0=glu_big[:, wx % RB, :],
                                scalar=1.0, in1=h_sb[:, wx, :],
                                op0=OP.mult, op1=OP.add,
                                accum_out=ssum[:, wx:wx + 1])
                    # sumsq per w0 (alternate DVE/ACT)
                    for wx in range(b0, wv + 1):
                        scr = cring.tile([128, D], BF, tag="scr")
                        if wx % 2 == 0:
                            nc.vector.scalar_tensor_tensor(
                                out=scr[:], in0=h_sb[:, wx, :], scalar=1.0,
                                in1=h_sb[:, wx, :], op0=OP.mult, op1=OP.mult,
                                accum_out=sqs[:, wx:wx + 1])
                        else:
                            nc.scalar.activation(
                                scr[:], h_sb[:, wx, :], AF.Square,
                                accum_out=sqs[:, wx:wx + 1])

            # ---- batched stats ----
            nc.vector.tensor_scalar(out=mu[:], in0=ssum[:],
                                    scalar1=1.0 / D, scalar2=None,
                                    op0=OP.mult)
            nc.vector.tensor_tensor(var[:], mu[:], mu[:], op=OP.mult)
            nc.vector.scalar_tensor_tensor(
                out=var[:], in0=sqs[:], scalar=1.0 / D,
                in1=var[:], op0=OP.mult, op1=OP.subtract)
            nc.scalar.activation(std[:], var[:], AF.Sqrt, bias=eps_t[:, 0:1])
            nc.vector.reciprocal(rr[:], std[:])
            nc.vector.scalar_tensor_tensor(
                out=nmr[:], in0=mu[:], scalar=-1.0,
                in1=rr[:], op0=OP.mult, op1=OP.mult)

            if flags["use_ln_affine"]:
                # fallback: eager per-w0 normalize + affine (unused when
                # ln is identity, which host_prep detects)
                for w0 in range(W):
                    nc.vector.tensor_scalar(
                        out=h_sb[:, w0, :], in0=h_sb[:, w0, :],
                        scalar1=rr[:, w0:w0 + 1], scalar2=nmr[:, w0:w0 + 1],
                        op0=OP.mult, op1=OP.add)
                    nc.vector.tensor_tensor(
                        h_sb[:, w0, :], h_sb[:, w0, :], lnw_t[:], op=OP.mult)
                    nc.vector.tensor_tensor(
                        h_sb[:, w0, :], h_sb[:, w0, :], lnb_t[:], op=OP.add)

        # ---------------- decoder ----------------
        dec_sb = consts.tile([128, W], F32)
        for w0 in range(W):
            scr = cring.tile([128, D], BF, tag="scr")
            nc.vector.scalar_tensor_tensor(
                out=scr[:], in0=h_sb[:, w0, :], scalar=1.0, in1=wdec_t[:],
                op0=OP.mult, op1=OP.mult, accum_out=dec_sb[:, w0:w0 + 1])
        if not flags["use_ln_affine"]:
            # h held pre-normalize p: out = dec*rr + nmr*sum(wdec)
            nc.vector.tensor_tensor(dec_sb[:], dec_sb[:], rr[:], op=OP.mult)
            nc.vector.scalar_tensor_tensor(
                out=dec_sb[:], in0=nmr[:], scalar=swd_t[:, 0:1], in1=dec_sb[:],
                op0=OP.mult, op1=OP.add)
        if flags["b_dec"] != 0.0:
            nc.vector.tensor_scalar(out=dec_sb[:], in0=dec_sb[:],
                                    scalar1=float(flags["b_dec"]), scalar2=None,
                                    op0=OP.add)
        nc.sync.dma_start(out[:], dec_sb[:])

    nc.compile()
    return nc


# ---------------------------------------------------------------------------
# Self-contained entry point: full inputs in, full output out.
# Shards batch-parallel across 8 NeuronCores (cores 4..7 duplicate work).
# ---------------------------------------------------------------------------

_PROGRAM_CACHE = {}


def _get_program(flags):
    key = (flags["n_layers"], flags["d_model"], flags["use_ln_affine"],
           flags["use_b_out"], flags["b_dec"])
    if key not in _PROGRAM_CACHE:
        _PROGRAM_CACHE[key] = build_program(flags, num_devices=8)
    return _PROGRAM_CACHE[key]


def kernel(**inputs):
    import os
    from concourse.bass_utils import run_bass_kernel_spmd

    common, per_batch, flags = host_prep(inputs)
    nc = _get_program(flags)

    B = len(per_batch)
    in_maps = []
    for c in range(8):
        m = dict(common)
        m.update(per_batch[c % B])
        in_maps.append(m)

    trace = bool(os.environ.get("S4ND_TRACE"))
    res = run_bass_kernel_spmd(nc, in_maps, core_ids=list(range(8)), trace=trace)
    if trace and res.exec_time_ns is not None:
        print(f"HW exec time: {res.exec_time_ns} ns")
        kernel.last_exec_time_ns = res.exec_time_ns
        kernel.last_results = res

    out = np.stack([res.results[b]["out"] for b in range(B)], axis=0)[..., None]
    return out.astype(np.float32)
